# revision 24
# baseline (speedup 1.0000x reference)
"""GAT 2-layer kernel for Trainium2, 8 NeuronCores.

Strategy (graph/data parallel, dst-sharded):
 - Host: sort edges by dst, pack per-core / per-dst-tile chunk streams
   (128 edges per chunk), fold attention vectors into the weight matrix so a
   single matmul produces per-node rows [h | a_src | a_dst].
 - Device, per layer: build T = x @ Wc (node feature table, bf16, in HBM,
   partition-major row order so builds write contiguously at full DMA rate),
   then per dst-tile: per-chunk indirect-DMA gathers of T[src] rows (the HW
   indirect path supports one index per partition per instruction), a_dst
   broadcast to edges via PE (transpose the one-hot S then a small matmul
   against the tile's a_dst rows - no per-edge dst gather), per-edge
   w = exp(leakyrelu(a_src+a_dst)) with leakyrelu as max(x, 0.2x) on DVE so
   the ACT engine only ever holds Exp, and aggregation of numerator +
   denominator with a selection-matrix matmul into PSUM. Outputs are staged
   in SBUF and written once per layer; layer-2 log_softmax defers Ln to a
   single final pass.
 - Two launches (layer1, layer2); host concatenates layer1 shards (the
   "all-to-all halo exchange" of the sharding hint).
"""

import numpy as np
import ml_dtypes
from contextlib import ExitStack

import concourse.bass as bass
import concourse.tile as tile
from concourse import bacc, mybir
from concourse.bass import ts, ds
from concourse.bass_utils import run_bass_kernel_spmd

BF16 = mybir.dt.bfloat16
F32 = mybir.dt.float32
I32 = mybir.dt.int32
NPBF16 = ml_dtypes.bfloat16

P = 128
NCORES = 8
N = 50000
E = 1600000
TPC = 49                      # dst tiles per core
G = NCORES * TPC              # 392 global tiles
NPAD = G * P                  # 50176 padded node count
NEG_SLOPE = 0.2
BG = 8                        # build tiles per group (G = 49*8)
B4 = 4                        # chunks per PSUM batch in the a_dst broadcast

import os as _os
TRACE = bool(_os.environ.get("KERNEL_TRACE"))
TRACE_DIR = _os.environ.get("KERNEL_TRACE_DIR") or None
LAST_RESULTS: list = []
LAST_NCS: list = []


def _prep_edges(edge_index):
    """Sort edges by dst; build per-core [P, NCH] streams with per-tile
    chunk padding shared across cores (SPMD static shapes).

    src stream addresses the partition-major table T4, where node
    n = t*128+p is stored at row p*G + t."""
    src = edge_index[0].astype(np.int64)
    dst = edge_index[1].astype(np.int64)
    order = np.argsort(dst, kind="stable")
    srcs = src[order]
    dsts = dst[order]

    tile_of_edge = dsts >> 7                       # [E]
    counts = np.bincount(tile_of_edge, minlength=G)
    cnt2 = counts.reshape(NCORES, TPC)
    CH = np.maximum((cnt2 + P - 1) // P, 1).max(axis=0).astype(np.int64)  # [TPC]
    cumCH = np.concatenate([[0], np.cumsum(CH)]).astype(np.int64)
    NCH = int(cumCH[-1])

    src_arr = np.zeros((NCORES, P, NCH), np.int32)
    dstf_arr = np.full((NCORES, P, NCH), -1.0, NPBF16)

    tile_starts = np.concatenate([[0], np.cumsum(counts)])
    rank = np.arange(E, dtype=np.int64) - tile_starts[tile_of_edge]
    core_of_edge = tile_of_edge // TPC
    ltile = tile_of_edge % TPC
    col = cumCH[ltile] + (rank >> 7)
    part = rank & 127

    src_arr[core_of_edge, part, col] = ((srcs & 127) * G + (srcs >> 7)
                                        ).astype(np.int32)
    dstf_arr[core_of_edge, part, col] = (dsts & 127).astype(NPBF16)

    # per-core row ids of this core's own dst tiles in the Tb4 flat table
    # (only column 0 is read on-device: rows p*G+k*TPC+t are consecutive in
    # t, so one 2D-out indirect DMA run-gathers all TPC tiles' a_dst rows)
    tbrow = np.empty((NCORES, P, TPC), np.int32)
    for k in range(NCORES):
        tbrow[k] = (np.arange(P)[:, None] * G
                    + (k * TPC + np.arange(TPC))[None, :])
    return src_arr, dstf_arr, tbrow, [int(c) for c in CH], cumCH


def _build_layer_program(KIN, F_G, F_D, CH_list, cumCH, NCH, layer):
    """One SPMD Bass program for one GAT layer.

    KIN: input feature dim (256 / 64); F_G: gathered row width (64+F_D),
    F_D: heads (8 / 1). Layer 1 outputs bf16 elu(...); layer 2 outputs f32
    log_softmax rows. Output layout [P, TPC, 64]: row (p, t) = node t*128+p.
    """
    F_H = 64
    WCW = F_G + F_D          # built table row width (h | a_src | a_dst)
    RW = F_H + F_D           # matmul rhs / psum width (msg | w)
    KT = (KIN + P - 1) // P  # K tiles for the build matmul
    KP = min(KIN, P)         # partition size of build lhsT
    CHmax = max(CH_list)
    out_dt = BF16 if layer == 1 else F32
    CW = F_H // F_D

    nc = bacc.Bacc("TRN2", target_bir_lowering=False, debug=False,
                   num_devices=NCORES)

    xT_in = nc.dram_tensor("xT", [KIN, NPAD], BF16, kind="ExternalInput").ap()
    wc_in = nc.dram_tensor("wc", [KIN, WCW], BF16, kind="ExternalInput").ap()
    src_in = nc.dram_tensor("srcs", [P, NCH], I32, kind="ExternalInput").ap()
    dstf_in = nc.dram_tensor("dstf", [P, NCH], BF16, kind="ExternalInput").ap()
    tbr_in = nc.dram_tensor("tbrow", [P, TPC], I32, kind="ExternalInput").ap()
    bias_in = nc.dram_tensor("bias", [1, F_H], F32, kind="ExternalInput").ap()
    out_dram = nc.dram_tensor("out", [P, TPC, F_H], out_dt,
                              kind="ExternalOutput").ap()

    with tile.TileContext(nc) as tc, ExitStack() as ctx:
        cpool = ctx.enter_context(tc.tile_pool(name="const", bufs=1))
        dpool = ctx.enter_context(tc.tile_pool(name="dram", bufs=1,
                                               space=bass.MemorySpace.DRAM))
        bpool = ctx.enter_context(tc.tile_pool(name="bld", bufs=2))
        stpool = ctx.enter_context(tc.tile_pool(name="bst", bufs=2))
        gpool = ctx.enter_context(tc.tile_pool(name="gat", bufs=2))
        spool = ctx.enter_context(tc.tile_pool(name="sel", bufs=2))
        s2pool = ctx.enter_context(tc.tile_pool(name="s2", bufs=3))
        epool = ctx.enter_context(tc.tile_pool(name="edge", bufs=2))
        opool = ctx.enter_context(tc.tile_pool(name="post", bufs=2))
        tbpool = ctx.enter_context(tc.tile_pool(name="tbt", bufs=1))
        pps = ctx.enter_context(tc.tile_pool(name="psb", bufs=2,
                                             space=bass.MemorySpace.PSUM))
        ppt = ctx.enter_context(tc.tile_pool(name="pst", bufs=2,
                                             space=bass.MemorySpace.PSUM))
        ppa = ctx.enter_context(tc.tile_pool(name="psa", bufs=2,
                                             space=bass.MemorySpace.PSUM))
        ppe = ctx.enter_context(tc.tile_pool(name="pse", bufs=2,
                                             space=bass.MemorySpace.PSUM))

        # ---- constants ----
        wc_sb = cpool.tile([KP, KT, WCW], BF16)
        for kt in range(KT):
            nc.sync.dma_start(wc_sb[:, kt, :], wc_in[kt * KP:(kt + 1) * KP, :])
        bias_sb = cpool.tile([P, F_H], F32)
        nc.sync.dma_start(bias_sb[:], bias_in.to_broadcast((P, F_H)))
        # iota over the d (middle) axis: value = d for all (p, d, c)
        iota_i = stpool.tile([P, P, CHmax], I32)
        nc.gpsimd.iota(iota_i[:], pattern=[[1, P], [0, CHmax]],
                       channel_multiplier=0)
        iota_f = cpool.tile([P, P, CHmax], BF16)
        nc.vector.tensor_copy(iota_f[:], iota_i[:])
        # identity for PE transposes
        idn_p = stpool.tile([P, P], I32)
        nc.gpsimd.iota(idn_p[:], pattern=[[0, P]], channel_multiplier=1)
        idn_f = stpool.tile([P, P], I32)
        nc.gpsimd.iota(idn_f[:], pattern=[[1, P]], channel_multiplier=0)
        ident = cpool.tile([P, P], BF16)
        nc.vector.tensor_tensor(ident[:], idn_p[:], idn_f[:],
                                op=mybir.AluOpType.is_equal)

        # ---- whole-layer edge streams (one DMA each) ----
        src_sb = cpool.tile([P, NCH], I32)
        nc.sync.dma_start(src_sb[:], src_in[:])
        dstf_sb = cpool.tile([P, NCH], BF16)
        nc.sync.dma_start(dstf_sb[:], dstf_in[:])
        tbr_sb = cpool.tile([P, TPC], I32)
        nc.sync.dma_start(tbr_sb[:], tbr_in[:])

        # ---- output staging ----
        ostage = cpool.tile([P, TPC, F_H], out_dt)
        if layer == 2:
            s_all = cpool.tile([P, TPC], F32)

        # ---- phase 1: build T4 = [h | a_src], Tb4 = [a_dst], both
        # partition-major: node t*128+p at row p*G+t ----
        T4 = dpool.tile([P, G, F_G], BF16)
        Tb4 = dpool.tile([P, G, F_D], BF16)
        T4flat = T4[:, :, :].rearrange("p g f -> (p g) f")
        Tb4flat = Tb4[:, :, :].rearrange("p g f -> (p g) f")
        for gi in range(G // BG):
            xt = bpool.tile([KP, KT, BG * P], BF16)
            for kt in range(KT):
                nc.sync.dma_start(
                    xt[:, kt, :],
                    xT_in[kt * KP:(kt + 1) * KP, gi * BG * P:(gi + 1) * BG * P])
            Tst = stpool.tile([P, BG, F_G], BF16)
            Tbst = stpool.tile([P, BG, F_D], BF16)
            for b0 in range(0, BG, B4):
                psB = pps.tile([P, B4, WCW], F32)
                for b in range(b0, b0 + B4):
                    for kt in range(KT):
                        nc.tensor.matmul(
                            psB[:, b - b0, :], xt[:, kt, b * P:(b + 1) * P],
                            wc_sb[:, kt, :],
                            start=(kt == 0), stop=(kt == KT - 1))
                nc.vector.tensor_copy(Tst[:, b0:b0 + B4, :],
                                      psB[:, :, 0:F_G])
                nc.vector.tensor_copy(Tbst[:, b0:b0 + B4, :],
                                      psB[:, :, F_G:WCW])
            nc.sync.dma_start(T4[:, gi * BG:(gi + 1) * BG, :], Tst[:])
            nc.sync.dma_start(Tb4[:, gi * BG:(gi + 1) * BG, :], Tbst[:])

        # ---- phase 2: per dst-tile edge aggregation ----
        # One run-gather pulls every own tile's a_dst rows: per partition p,
        # Tb4flat rows p*G + k*TPC + t are consecutive for t in [0, TPC), and
        # a 2D-out indirect DMA gathers out-row-width bytes contiguously from
        # each partition's indexed row (HW requires 2D out; 3D outs break).
        tb_all = tbpool.tile([P, TPC * F_D], BF16)
        nc.gpsimd.indirect_dma_start(
            out=tb_all[:], out_offset=None, in_=Tb4flat,
            in_offset=bass.IndirectOffsetOnAxis(
                ap=tbr_sb[:, 0:1], axis=0))
        for t in range(TPC):
            CH = CH_list[t]
            c0 = int(cumCH[t])
            tb_t = tb_all[:, t * F_D:(t + 1) * F_D]
            # selection one-hot S[e, d, c] = (dst_local[e, c] == d)
            S_t = spool.tile([P, P, CHmax], BF16)
            nc.vector.tensor_tensor(
                S_t[:, :, 0:CH],
                dstf_sb[:, c0:c0 + CH].unsqueeze(1).to_broadcast((P, P, CH)),
                iota_f[:, :, 0:CH], op=mybir.AluOpType.is_equal)
            # gather G rows per chunk (HW: one index per partition)
            G_t = gpool.tile([P, CHmax, F_G], BF16)
            for c in range(CH):
                nc.gpsimd.indirect_dma_start(
                    out=G_t[:, c, :], out_offset=None, in_=T4flat,
                    in_offset=bass.IndirectOffsetOnAxis(
                        ap=src_sb[:, c0 + c:c0 + c + 1], axis=0))
            # a_dst broadcast to edges: D[e] = (S^T)^T tb = S tb, via PE
            D_t = epool.tile([P, CHmax, F_D], BF16)
            for b0 in range(0, CH, B4):
                n = min(B4, CH - b0)
                psTR = ppt.tile([P, B4, P], BF16)
                for j in range(n):
                    nc.tensor.transpose(psTR[:, j, :], S_t[:, :, b0 + j],
                                        ident[:])
                S2_t = s2pool.tile([P, B4, P], BF16)
                nc.vector.tensor_copy(S2_t[:, 0:n, :], psTR[:, 0:n, :])
                psAD = ppa.tile([P, B4, F_D], F32)
                for j in range(n):
                    nc.tensor.matmul(psAD[:, j, :], S2_t[:, j, :], tb_t,
                                     start=True, stop=True)
                nc.vector.tensor_copy(D_t[:, b0:b0 + n, :], psAD[:, 0:n, :])

            # edge logits -> w = exp(max(s, 0.2s)) (= exp(leakyrelu))
            L_t = epool.tile([P, CHmax, F_D], F32)
            nc.vector.tensor_add(L_t[:, 0:CH, :], G_t[:, 0:CH, F_H:F_G],
                                 D_t[:, 0:CH, :])
            L2_t = epool.tile([P, CHmax, F_D], F32)
            nc.vector.tensor_scalar_mul(L2_t[:, 0:CH, :], L_t[:, 0:CH, :],
                                        NEG_SLOPE)
            nc.vector.tensor_tensor(L2_t[:, 0:CH, :], L_t[:, 0:CH, :],
                                    L2_t[:, 0:CH, :], op=mybir.AluOpType.max)
            rhs_t = epool.tile([P, CHmax, RW], BF16)
            nc.scalar.activation(rhs_t[:, 0:CH, F_H:RW], L2_t[:, 0:CH, :],
                                 mybir.ActivationFunctionType.Exp)
            # msg = w * h[src] (one 4D op; w broadcast across channels)
            nc.vector.tensor_mul(
                rhs_t[:, 0:CH, 0:F_H].rearrange(
                    "p c (h w) -> p c h w", h=F_D),
                G_t[:, 0:CH, 0:F_H].rearrange(
                    "p c (h w) -> p c h w", h=F_D),
                rhs_t[:, 0:CH, F_H:RW].unsqueeze(3).to_broadcast(
                    (P, CH, F_D, CW)))

            psE = ppe.tile([P, RW], F32)
            for c in range(CH):
                nc.tensor.matmul(psE[:], S_t[:, :, c], rhs_t[:, c, :],
                                 start=(c == 0), stop=(c == CH - 1))

            # ---- postprocess this dst-tile ----
            if layer == 1:
                den = opool.tile([P, F_D], F32)
                nc.vector.tensor_scalar_add(den[:], psE[:, F_H:RW], 1e-16)
                rec = opool.tile([P, F_D], F32)
                nc.vector.reciprocal(rec[:], den[:])
                o1 = opool.tile([P, F_H], F32)
                nc.vector.tensor_mul(
                    o1[:].rearrange("p (h w) -> p h w", h=F_D),
                    psE[:, 0:F_H].rearrange("p (h w) -> p h w", h=F_D),
                    rec[:].unsqueeze(2).to_broadcast((P, F_D, CW)))
                nc.vector.tensor_add(o1[:], o1[:], bias_sb[:])
                # elu(x) = max(x,0) + exp(min(x,0)) - 1
                mn = opool.tile([P, F_H], F32)
                nc.vector.tensor_scalar_min(mn[:], o1[:], 0.0)
                em = opool.tile([P, F_H], F32)
                nc.scalar.activation(em[:], mn[:],
                                     mybir.ActivationFunctionType.Exp)
                mx = opool.tile([P, F_H], F32)
                nc.vector.tensor_scalar_max(mx[:], o1[:], 0.0)
                s1 = opool.tile([P, F_H], F32)
                nc.vector.tensor_add(s1[:], mx[:], em[:])
                nc.vector.tensor_scalar_add(ostage[:, t, :], s1[:], -1.0)
            else:
                den = opool.tile([P, 1], F32)
                nc.vector.tensor_scalar_add(den[:], psE[:, F_H:RW], 1e-16)
                rec = opool.tile([P, 1], F32)
                nc.vector.reciprocal(rec[:], den[:])
                o2 = opool.tile([P, F_H], F32)
                nc.vector.tensor_mul(
                    o2[:], psE[:, 0:F_H], rec[:].to_broadcast((P, F_H)))
                nc.vector.tensor_add(o2[:], o2[:], bias_sb[:])
                rm = opool.tile([P, 1], F32)
                nc.vector.tensor_reduce(rm[:], o2[:], mybir.AxisListType.X,
                                        mybir.AluOpType.max)
                nc.vector.tensor_tensor(ostage[:, t, :], o2[:],
                                        rm[:].to_broadcast((P, F_H)),
                                        op=mybir.AluOpType.subtract)
                e_t = opool.tile([P, F_H], F32)
                nc.scalar.activation(e_t[:], ostage[:, t, :],
                                     mybir.ActivationFunctionType.Exp,
                                     accum_out=s_all[:, t:t + 1])

        if layer == 2:
            # deferred log-softmax denominator: one Ln pass + one subtract
            ls = cpool.tile([P, TPC], F32)
            nc.scalar.activation(ls[:], s_all[:],
                                 mybir.ActivationFunctionType.Ln)
            nc.vector.tensor_tensor(
                ostage[:], ostage[:],
                ls[:].unsqueeze(2).to_broadcast((P, TPC, F_H)),
                op=mybir.AluOpType.subtract)
        nc.sync.dma_start(out_dram[:], ostage[:])

    nc.compile()
    LAST_NCS.append(nc)
    return nc


def _fold_weights1(W1, att_src1, att_dst1):
    A1s = np.zeros((64, 8), np.float32)
    A1s[np.arange(64), np.arange(64) // 8] = att_src1.reshape(64)
    A1d = np.zeros((64, 8), np.float32)
    A1d[np.arange(64), np.arange(64) // 8] = att_dst1.reshape(64)
    return np.concatenate([W1, W1 @ A1s, W1 @ A1d], axis=1)  # [256, 80]


def kernel(x, edge_index, W1, att_src1, att_dst1, bias1,
           W2, att_src2, att_dst2, bias2):
    x, edge_index = np.asarray(x), np.asarray(edge_index)
    W1, att_src1 = np.asarray(W1), np.asarray(att_src1)
    att_dst1, bias1 = np.asarray(att_dst1), np.asarray(bias1)
    W2, att_src2 = np.asarray(W2), np.asarray(att_src2)
    att_dst2, bias2 = np.asarray(att_dst2), np.asarray(bias2)
    LAST_RESULTS.clear()
    LAST_NCS.clear()
    src_arr, dstf_arr, tbrow, CH_list, cumCH = _prep_edges(edge_index)
    NCH = int(cumCH[-1])

    # ---------- layer 1 ----------
    Wc1 = _fold_weights1(W1, att_src1, att_dst1).astype(NPBF16)
    xT = np.zeros((256, NPAD), NPBF16)
    xT[:, :N] = x.T.astype(NPBF16)

    nc1 = _build_layer_program(256, 72, 8, CH_list, cumCH, NCH, layer=1)
    in_maps = [{
        "xT": xT, "wc": Wc1,
        "srcs": np.ascontiguousarray(src_arr[k]),
        "dstf": np.ascontiguousarray(dstf_arr[k]),
        "tbrow": np.ascontiguousarray(tbrow[k]),
        "bias": bias1.astype(np.float32).reshape(1, 64),
    } for k in range(NCORES)]
    kw1 = {}
    if TRACE:
        kw1 = dict(trace=True,
                   tmpdir=(TRACE_DIR + "/l1") if TRACE_DIR else None)
        if kw1["tmpdir"]:
            _os.makedirs(kw1["tmpdir"], exist_ok=True)
    res1 = run_bass_kernel_spmd(nc1, in_maps, core_ids=list(range(NCORES)),
                                **kw1)
    LAST_RESULTS.append(res1)
    # out[k] is [P, TPC, 64], row (p, t) = node (k*TPC+t)*128+p.
    # Assemble x2T [64, NPAD] with node index ((k*TPC+t)*128+p).
    big = np.stack([res1.results[k]["out"] for k in range(NCORES)])
    x2T = np.ascontiguousarray(
        big.transpose(3, 0, 2, 1).reshape(64, NPAD)).astype(NPBF16)

    # ---------- layer 2 ----------
    Wc2 = np.concatenate(
        [W2, W2 @ att_src2.T, W2 @ att_dst2.T], axis=1).astype(NPBF16)

    nc2 = _build_layer_program(64, 65, 1, CH_list, cumCH, NCH, layer=2)
    in_maps2 = [{
        "xT": x2T, "wc": Wc2,
        "srcs": np.ascontiguousarray(src_arr[k]),
        "dstf": np.ascontiguousarray(dstf_arr[k]),
        "tbrow": np.ascontiguousarray(tbrow[k]),
        "bias": bias2.astype(np.float32).reshape(1, 64),
    } for k in range(NCORES)]
    kw2 = {}
    if TRACE:
        kw2 = dict(trace=True,
                   tmpdir=(TRACE_DIR + "/l2") if TRACE_DIR else None)
        if kw2["tmpdir"]:
            _os.makedirs(kw2["tmpdir"], exist_ok=True)
    res2 = run_bass_kernel_spmd(nc2, in_maps2, core_ids=list(range(NCORES)),
                                **kw2)
    LAST_RESULTS.append(res2)
    out = np.stack([res2.results[k]["out"] for k in range(NCORES)])
    out = out.transpose(0, 2, 1, 3).reshape(NPAD, 64)
    return np.ascontiguousarray(out[:N]).astype(np.float32)


# revision 29
# speedup vs baseline: 1.0250x; 1.0250x over previous
"""GAT 2-layer kernel for Trainium2, 8 NeuronCores.

Strategy (graph/data parallel, dst-sharded):
 - Host: sort edges by dst, pack per-core / per-dst-tile chunk streams
   (128 edges per chunk), fold attention vectors into the weight matrix so a
   single matmul produces per-node rows [h | a_src | a_dst].
 - Device, per layer: build T = x @ Wc (node feature table, bf16, in HBM,
   partition-major row order so builds write contiguously at full DMA rate),
   then per dst-tile: per-chunk indirect-DMA gathers of T[src] rows (the HW
   indirect path supports one index per partition per instruction), a_dst
   broadcast to edges via PE (transpose the one-hot S then a small matmul
   against the tile's a_dst rows - no per-edge dst gather), per-edge
   w = exp(leakyrelu(a_src+a_dst)) with leakyrelu as max(x, 0.2x) on DVE so
   the ACT engine only ever holds Exp, and aggregation of numerator +
   denominator with a selection-matrix matmul into PSUM. Outputs are staged
   in SBUF and written once per layer; layer-2 log_softmax defers Ln to a
   single final pass.
 - Two launches (layer1, layer2); host concatenates layer1 shards (the
   "all-to-all halo exchange" of the sharding hint).
"""

import numpy as np
import ml_dtypes
from contextlib import ExitStack

import concourse.bass as bass
import concourse.tile as tile
from concourse import bacc, mybir
from concourse.bass import ts, ds
from concourse.bass_utils import run_bass_kernel_spmd

BF16 = mybir.dt.bfloat16
F32 = mybir.dt.float32
I32 = mybir.dt.int32
NPBF16 = ml_dtypes.bfloat16

P = 128
NCORES = 8
N = 50000
E = 1600000
TPC = 49                      # dst tiles per core
G = NCORES * TPC              # 392 global tiles
NPAD = G * P                  # 50176 padded node count
NEG_SLOPE = 0.2
BG = 28                       # build tiles per group (G = 14*28)
B4 = 4                        # chunks per PSUM batch in the a_dst broadcast

import os as _os
TRACE = bool(_os.environ.get("KERNEL_TRACE"))
TRACE_DIR = _os.environ.get("KERNEL_TRACE_DIR") or None
LAST_RESULTS: list = []
LAST_NCS: list = []


def _prep_edges(edge_index):
    """Sort edges by dst; per (core, tile) split edges into PAIRS (src s and
    s+128 with (s>>7) even - adjacent partition-major table rows, fetched two
    rows per index by one 2D-out indirect DMA) and SINGLES. Streams:
      srcg [P, NG]: one anchor table-row per gather instruction column
        (per tile: PCH pair columns then SCH single columns)
      dstf [P, NS]: dst-local per slot column (per tile: 2*PCH pair slots
        then SCH single slots); -1 pads."""
    src = edge_index[0].astype(np.int64)
    dst = edge_index[1].astype(np.int64)
    # sort by (dst-tile, src) so (tile, src) groups are contiguous
    gt = dst >> 7
    ordr = np.lexsort((src, gt))
    srcs = src[ordr]
    dsts = dst[ordr]
    gte = gt[ordr]

    key = gte * NPAD + srcs
    uk, ust, uc = np.unique(key, return_index=True, return_counts=True)
    ug = uk // NPAD
    us = uk % NPAD
    # partner groups: (g, s) even src-tile -> (g, s+128)
    pk = uk + 128
    pidx = np.searchsorted(uk, pk)
    pidx_c = np.clip(pidx, 0, len(uk) - 1)
    has = (uk[pidx_c] == pk) & (((us >> 7) & 1) == 0) & ((us >> 7) < G - 1)
    npf = np.zeros(len(uk), np.int64)            # pairs where u is FIRST
    npf[has] = np.minimum(uc[has], uc[pidx_c[has]])
    nps = np.zeros(len(uk), np.int64)            # pairs where u is SECOND
    nps[pidx_c[has]] = npf[has]

    E_ = len(srcs)
    grp = np.repeat(np.arange(len(uk)), uc)
    rank_in_grp = np.arange(E_) - ust[grp]
    is_first = rank_in_grp < npf[grp]
    is_second = rank_in_grp < nps[grp]
    is_single = ~(is_first | is_second)

    # per-tile pair counts and per-core/per-tile singles
    tile_of_u = ug
    pairs_per_gt = np.bincount(tile_of_u, weights=npf, minlength=G).astype(
        np.int64)
    cnt_per_gt = np.bincount(gte, minlength=G)
    sing_per_gt = cnt_per_gt - 2 * pairs_per_gt
    pcg = pairs_per_gt.reshape(NCORES, TPC)
    scg = sing_per_gt.reshape(NCORES, TPC)
    PCH = np.maximum((pcg + P - 1) // P, 0).max(axis=0).astype(np.int64)
    SCH = np.maximum((scg + P - 1) // P, 1).max(axis=0).astype(np.int64)
    CHS = 2 * PCH + SCH                     # slot columns per tile
    GCH = PCH + SCH                         # gather columns per tile
    cumS = np.concatenate([[0], np.cumsum(CHS)]).astype(np.int64)
    cumG = np.concatenate([[0], np.cumsum(GCH)]).astype(np.int64)
    NS = int(cumS[-1])
    NG = int(cumG[-1])

    # tile-local pair index q for firsts: offset of group within tile + rank
    poff_u = np.zeros(len(uk), np.int64)
    # cumsum of npf within each tile
    cs = np.cumsum(npf) - npf
    tile_first_u = np.searchsorted(tile_of_u, np.arange(G), side="left")
    base_of_tile_u = np.zeros(G, np.int64)
    valid = tile_first_u < len(uk)
    base_of_tile_u[valid] = cs[tile_first_u[valid]]
    poff_u = cs - base_of_tile_u[tile_of_u]
    q_first = poff_u[grp] + rank_in_grp           # valid where is_first
    # seconds: q equals the FIRST-side q of the partner group
    poff_partner = np.full(len(uk), -1, np.int64)
    poff_partner[pidx_c[has]] = poff_u[has]
    q_second = poff_partner[grp] + rank_in_grp    # valid where is_second

    # tile-local single rank
    sing_cum = np.cumsum(is_single) - is_single
    tile_edge_start = np.concatenate([[0], np.cumsum(cnt_per_gt)])
    sbase = np.zeros(E_, np.int64)
    sbase = sing_cum - (sing_cum[tile_edge_start[gte]] -
                        is_single[tile_edge_start[gte]] * 0)
    r_single = sing_cum - sing_cum[tile_edge_start[gte]]

    core = gte // TPC
    lt = gte % TPC
    row_of = (srcs & 127) * G + (srcs >> 7)

    src_arr = np.zeros((NCORES, P, NG), np.int32)
    dstf_arr = np.full((NCORES, P, NS), -1.0, NPBF16)

    # firsts: gather col cumG[lt]+q>>7 anchor; slots (q&127, cumS+2*(q>>7))
    m = is_first
    j = q_first[m] >> 7
    p = q_first[m] & 127
    src_arr[core[m], p, cumG[lt[m]] + j] = row_of[m].astype(np.int32)
    dstf_arr[core[m], p, cumS[lt[m]] + 2 * j] = (dsts[m] & 127).astype(NPBF16)
    # seconds: slot col +1 (no separate gather col)
    m = is_second
    j = q_second[m] >> 7
    p = q_second[m] & 127
    dstf_arr[core[m], p, cumS[lt[m]] + 2 * j + 1] = (
        dsts[m] & 127).astype(NPBF16)
    # singles: gather col cumG+PCH+r>>7; slot col cumS+2*PCH+r>>7
    m = is_single
    j = r_single[m] >> 7
    p = r_single[m] & 127
    src_arr[core[m], p, cumG[lt[m]] + PCH[lt[m]] + j] = row_of[m].astype(
        np.int32)
    dstf_arr[core[m], p, cumS[lt[m]] + 2 * PCH[lt[m]] + j] = (
        dsts[m] & 127).astype(NPBF16)

    tbrow = np.empty((NCORES, P, TPC), np.int32)
    for k in range(NCORES):
        tbrow[k] = (np.arange(P)[:, None] * G
                    + (k * TPC + np.arange(TPC))[None, :])
    return (src_arr, dstf_arr, tbrow, [int(c) for c in PCH],
            [int(c) for c in SCH], cumS, cumG)


def _build_layer_program(KIN, F_G, F_D, PCH_list, SCH_list, cumS, cumG,
                         layer):
    """One SPMD Bass program for one GAT layer.

    KIN: input feature dim (256 / 64); F_G: gathered row width (64+F_D),
    F_D: heads (8 / 1). Layer 1 outputs bf16 elu(...); layer 2 outputs f32
    log_softmax rows. Output layout [P, TPC, 64]: row (p, t) = node t*128+p.
    """
    F_H = 64
    WCW = F_G + F_D          # built table row width (h | a_src | a_dst)
    RW = F_H + F_D           # matmul rhs / psum width (msg | w)
    KT = (KIN + P - 1) // P  # K tiles for the build matmul
    KP = min(KIN, P)         # partition size of build lhsT
    NS = int(cumS[-1])
    NG = int(cumG[-1])
    CHmax = max(2 * p + s for p, s in zip(PCH_list, SCH_list))
    out_dt = BF16 if layer == 1 else F32
    CW = F_H // F_D

    nc = bacc.Bacc("TRN2", target_bir_lowering=False, debug=False,
                   num_devices=NCORES)

    xT_in = nc.dram_tensor("xT", [KIN, NPAD], BF16, kind="ExternalInput").ap()
    wc_in = nc.dram_tensor("wc", [KIN, WCW], BF16, kind="ExternalInput").ap()
    src_in = nc.dram_tensor("srcs", [P, NG], I32, kind="ExternalInput").ap()
    dstf_in = nc.dram_tensor("dstf", [P, NS], BF16, kind="ExternalInput").ap()
    tbr_in = nc.dram_tensor("tbrow", [P, TPC], I32, kind="ExternalInput").ap()
    bias_in = nc.dram_tensor("bias", [1, F_H], F32, kind="ExternalInput").ap()
    out_dram = nc.dram_tensor("out", [P, TPC, F_H], out_dt,
                              kind="ExternalOutput").ap()

    with tile.TileContext(nc) as tc, ExitStack() as ctx:
        cpool = ctx.enter_context(tc.tile_pool(name="const", bufs=1))
        dpool = ctx.enter_context(tc.tile_pool(name="dram", bufs=1,
                                               space=bass.MemorySpace.DRAM))
        bpool = ctx.enter_context(tc.tile_pool(name="bld", bufs=2))
        stpool = ctx.enter_context(tc.tile_pool(name="bst", bufs=2))
        gpool = ctx.enter_context(tc.tile_pool(name="gat", bufs=2))
        spool = ctx.enter_context(tc.tile_pool(name="sel", bufs=2))
        s2pool = ctx.enter_context(tc.tile_pool(name="s2", bufs=3))
        epool = ctx.enter_context(tc.tile_pool(name="edge", bufs=2))
        opool = ctx.enter_context(tc.tile_pool(name="post", bufs=2))
        tbpool = ctx.enter_context(tc.tile_pool(name="tbt", bufs=1))
        pps = ctx.enter_context(tc.tile_pool(name="psb", bufs=2,
                                             space=bass.MemorySpace.PSUM))
        ppt = ctx.enter_context(tc.tile_pool(name="pst", bufs=2,
                                             space=bass.MemorySpace.PSUM))
        ppa = ctx.enter_context(tc.tile_pool(name="psa", bufs=2,
                                             space=bass.MemorySpace.PSUM))
        ppe = ctx.enter_context(tc.tile_pool(name="pse", bufs=2,
                                             space=bass.MemorySpace.PSUM))

        # ---- constants ----
        wc_sb = cpool.tile([KP, KT, WCW], BF16)
        for kt in range(KT):
            nc.sync.dma_start(wc_sb[:, kt, :], wc_in[kt * KP:(kt + 1) * KP, :])
        bias_sb = cpool.tile([P, F_H], F32)
        nc.sync.dma_start(bias_sb[:], bias_in.to_broadcast((P, F_H)))
        # iota over the d (middle) axis: value = d for all (p, d, c)
        iota_i = stpool.tile([P, P, CHmax], I32)
        nc.gpsimd.iota(iota_i[:], pattern=[[1, P], [0, CHmax]],
                       channel_multiplier=0)
        iota_f = cpool.tile([P, P, CHmax], BF16)
        nc.vector.tensor_copy(iota_f[:], iota_i[:])
        # identity for PE transposes
        idn_p = stpool.tile([P, P], I32)
        nc.gpsimd.iota(idn_p[:], pattern=[[0, P]], channel_multiplier=1)
        idn_f = stpool.tile([P, P], I32)
        nc.gpsimd.iota(idn_f[:], pattern=[[1, P]], channel_multiplier=0)
        ident = cpool.tile([P, P], BF16)
        nc.vector.tensor_tensor(ident[:], idn_p[:], idn_f[:],
                                op=mybir.AluOpType.is_equal)

        # ---- whole-layer edge streams (one DMA each) ----
        src_sb = cpool.tile([P, NG], I32)
        nc.sync.dma_start(src_sb[:], src_in[:])
        dstf_sb = cpool.tile([P, NS], BF16)
        nc.sync.dma_start(dstf_sb[:], dstf_in[:])
        tbr_sb = cpool.tile([P, TPC], I32)
        nc.sync.dma_start(tbr_sb[:], tbr_in[:])

        # ---- output staging ----
        ostage = cpool.tile([P, TPC, F_H], out_dt)
        if layer == 2:
            s_all = cpool.tile([P, TPC], F32)

        # ---- phase 1: build T4 = [h | a_src], Tb4 = [a_dst], both
        # partition-major: node t*128+p at row p*G+t ----
        T4 = dpool.tile([P, G, F_G], BF16)
        Tb4 = dpool.tile([P, G, F_D], BF16)
        T4flat = T4[:, :, :].rearrange("p g f -> (p g) f")
        Tb4flat = Tb4[:, :, :].rearrange("p g f -> (p g) f")
        for gi in range(G // BG):
            xt = bpool.tile([KP, KT, BG * P], BF16)
            for kt in range(KT):
                nc.sync.dma_start(
                    xt[:, kt, :],
                    xT_in[kt * KP:(kt + 1) * KP, gi * BG * P:(gi + 1) * BG * P])
            Tst = stpool.tile([P, BG, F_G], BF16)
            Tbst = stpool.tile([P, BG, F_D], BF16)
            for b0 in range(0, BG, B4):
                nb = min(B4, BG - b0)
                psB = pps.tile([P, B4, WCW], F32)
                for b in range(b0, b0 + nb):
                    for kt in range(KT):
                        nc.tensor.matmul(
                            psB[:, b - b0, :], xt[:, kt, b * P:(b + 1) * P],
                            wc_sb[:, kt, :],
                            start=(kt == 0), stop=(kt == KT - 1))
                nc.vector.tensor_copy(Tst[:, b0:b0 + nb, :],
                                      psB[:, 0:nb, 0:F_G])
                nc.vector.tensor_copy(Tbst[:, b0:b0 + nb, :],
                                      psB[:, 0:nb, F_G:WCW])
            nc.sync.dma_start(T4[:, gi * BG:(gi + 1) * BG, :], Tst[:])
            nc.sync.dma_start(Tb4[:, gi * BG:(gi + 1) * BG, :], Tbst[:])

        # ---- phase 2: per dst-tile edge aggregation ----
        # One run-gather pulls every own tile's a_dst rows: per partition p,
        # Tb4flat rows p*G + k*TPC + t are consecutive for t in [0, TPC), and
        # a 2D-out indirect DMA gathers out-row-width bytes contiguously from
        # each partition's indexed row (HW requires 2D out; 3D outs break).
        tb_all = tbpool.tile([P, TPC * F_D], BF16)
        nc.gpsimd.indirect_dma_start(
            out=tb_all[:], out_offset=None, in_=Tb4flat,
            in_offset=bass.IndirectOffsetOnAxis(
                ap=tbr_sb[:, 0:1], axis=0))
        for t in range(TPC):
            PCH, SCH = PCH_list[t], SCH_list[t]
            CH = 2 * PCH + SCH
            c0 = int(cumS[t])
            g0 = int(cumG[t])
            tb_t = tb_all[:, t * F_D:(t + 1) * F_D]
            # selection one-hot S[e, d, c] = (dst_local[e, c] == d)
            S_t = spool.tile([P, P, CHmax], BF16)
            nc.vector.tensor_tensor(
                S_t[:, :, 0:CH],
                dstf_sb[:, c0:c0 + CH].unsqueeze(1).to_broadcast((P, P, CH)),
                iota_f[:, :, 0:CH], op=mybir.AluOpType.is_equal)
            # gather G rows (HW: one index per partition per instruction;
            # pair columns fetch 2 consecutive table rows into 2 slot cols)
            G_t = gpool.tile([P, CHmax, F_G], BF16)
            G_t2 = G_t[:, :, :].rearrange("p c f -> p (c f)")
            for j in range(PCH):
                nc.gpsimd.indirect_dma_start(
                    out=G_t2[:, 2 * j * F_G:(2 * j + 2) * F_G],
                    out_offset=None, in_=T4flat,
                    in_offset=bass.IndirectOffsetOnAxis(
                        ap=src_sb[:, g0 + j:g0 + j + 1], axis=0))
            for c in range(SCH):
                nc.gpsimd.indirect_dma_start(
                    out=G_t[:, 2 * PCH + c, :], out_offset=None, in_=T4flat,
                    in_offset=bass.IndirectOffsetOnAxis(
                        ap=src_sb[:, g0 + PCH + c:g0 + PCH + c + 1], axis=0))
            # a_dst broadcast to edges: D[e] = (S^T)^T tb = S tb, via PE
            D_t = epool.tile([P, CHmax, F_D], BF16)
            for b0 in range(0, CH, B4):
                n = min(B4, CH - b0)
                psTR = ppt.tile([P, B4, P], BF16)
                for j in range(n):
                    nc.tensor.transpose(psTR[:, j, :], S_t[:, :, b0 + j],
                                        ident[:])
                S2_t = s2pool.tile([P, B4, P], BF16)
                nc.vector.tensor_copy(S2_t[:, 0:n, :], psTR[:, 0:n, :])
                psAD = ppa.tile([P, B4, F_D], F32)
                for j in range(n):
                    nc.tensor.matmul(psAD[:, j, :], S2_t[:, j, :], tb_t,
                                     start=True, stop=True)
                nc.vector.tensor_copy(D_t[:, b0:b0 + n, :], psAD[:, 0:n, :])

            # edge logits -> w = exp(max(s, 0.2s)) (= exp(leakyrelu))
            L_t = epool.tile([P, CHmax, F_D], F32)
            nc.vector.tensor_add(L_t[:, 0:CH, :], G_t[:, 0:CH, F_H:F_G],
                                 D_t[:, 0:CH, :])
            L2_t = epool.tile([P, CHmax, F_D], F32)
            nc.vector.tensor_scalar_mul(L2_t[:, 0:CH, :], L_t[:, 0:CH, :],
                                        NEG_SLOPE)
            nc.vector.tensor_tensor(L2_t[:, 0:CH, :], L_t[:, 0:CH, :],
                                    L2_t[:, 0:CH, :], op=mybir.AluOpType.max)
            rhs_t = epool.tile([P, CHmax, RW], BF16)
            nc.scalar.activation(rhs_t[:, 0:CH, F_H:RW], L2_t[:, 0:CH, :],
                                 mybir.ActivationFunctionType.Exp)
            # msg = w * h[src] (one 4D op; w broadcast across channels)
            nc.vector.tensor_mul(
                rhs_t[:, 0:CH, 0:F_H].rearrange(
                    "p c (h w) -> p c h w", h=F_D),
                G_t[:, 0:CH, 0:F_H].rearrange(
                    "p c (h w) -> p c h w", h=F_D),
                rhs_t[:, 0:CH, F_H:RW].unsqueeze(3).to_broadcast(
                    (P, CH, F_D, CW)))

            psE = ppe.tile([P, RW], F32)
            for c in range(CH):
                nc.tensor.matmul(psE[:], S_t[:, :, c], rhs_t[:, c, :],
                                 start=(c == 0), stop=(c == CH - 1))

            # ---- postprocess this dst-tile ----
            if layer == 1:
                den = opool.tile([P, F_D], F32)
                nc.vector.tensor_scalar_add(den[:], psE[:, F_H:RW], 1e-16)
                rec = opool.tile([P, F_D], F32)
                nc.vector.reciprocal(rec[:], den[:])
                o1 = opool.tile([P, F_H], F32)
                nc.vector.tensor_mul(
                    o1[:].rearrange("p (h w) -> p h w", h=F_D),
                    psE[:, 0:F_H].rearrange("p (h w) -> p h w", h=F_D),
                    rec[:].unsqueeze(2).to_broadcast((P, F_D, CW)))
                nc.vector.tensor_add(o1[:], o1[:], bias_sb[:])
                # elu(x) = max(x,0) + exp(min(x,0)) - 1
                mn = opool.tile([P, F_H], F32)
                nc.vector.tensor_scalar_min(mn[:], o1[:], 0.0)
                em = opool.tile([P, F_H], F32)
                nc.scalar.activation(em[:], mn[:],
                                     mybir.ActivationFunctionType.Exp)
                mx = opool.tile([P, F_H], F32)
                nc.vector.tensor_scalar_max(mx[:], o1[:], 0.0)
                s1 = opool.tile([P, F_H], F32)
                nc.vector.tensor_add(s1[:], mx[:], em[:])
                nc.vector.tensor_scalar_add(ostage[:, t, :], s1[:], -1.0)
            else:
                den = opool.tile([P, 1], F32)
                nc.vector.tensor_scalar_add(den[:], psE[:, F_H:RW], 1e-16)
                rec = opool.tile([P, 1], F32)
                nc.vector.reciprocal(rec[:], den[:])
                o2 = opool.tile([P, F_H], F32)
                nc.vector.tensor_mul(
                    o2[:], psE[:, 0:F_H], rec[:].to_broadcast((P, F_H)))
                nc.vector.tensor_add(o2[:], o2[:], bias_sb[:])
                rm = opool.tile([P, 1], F32)
                nc.vector.tensor_reduce(rm[:], o2[:], mybir.AxisListType.X,
                                        mybir.AluOpType.max)
                nc.vector.tensor_tensor(ostage[:, t, :], o2[:],
                                        rm[:].to_broadcast((P, F_H)),
                                        op=mybir.AluOpType.subtract)
                e_t = opool.tile([P, F_H], F32)
                nc.scalar.activation(e_t[:], ostage[:, t, :],
                                     mybir.ActivationFunctionType.Exp,
                                     accum_out=s_all[:, t:t + 1])

        if layer == 2:
            # deferred log-softmax denominator: one Ln pass + one subtract
            ls = cpool.tile([P, TPC], F32)
            nc.scalar.activation(ls[:], s_all[:],
                                 mybir.ActivationFunctionType.Ln)
            nc.vector.tensor_tensor(
                ostage[:], ostage[:],
                ls[:].unsqueeze(2).to_broadcast((P, TPC, F_H)),
                op=mybir.AluOpType.subtract)
        nc.sync.dma_start(out_dram[:], ostage[:])

    nc.compile()
    LAST_NCS.append(nc)
    return nc


def _fold_weights1(W1, att_src1, att_dst1):
    A1s = np.zeros((64, 8), np.float32)
    A1s[np.arange(64), np.arange(64) // 8] = att_src1.reshape(64)
    A1d = np.zeros((64, 8), np.float32)
    A1d[np.arange(64), np.arange(64) // 8] = att_dst1.reshape(64)
    return np.concatenate([W1, W1 @ A1s, W1 @ A1d], axis=1)  # [256, 80]


def kernel(x, edge_index, W1, att_src1, att_dst1, bias1,
           W2, att_src2, att_dst2, bias2):
    x, edge_index = np.asarray(x), np.asarray(edge_index)
    W1, att_src1 = np.asarray(W1), np.asarray(att_src1)
    att_dst1, bias1 = np.asarray(att_dst1), np.asarray(bias1)
    W2, att_src2 = np.asarray(W2), np.asarray(att_src2)
    att_dst2, bias2 = np.asarray(att_dst2), np.asarray(bias2)
    LAST_RESULTS.clear()
    LAST_NCS.clear()
    (src_arr, dstf_arr, tbrow, PCH_list, SCH_list,
     cumS, cumG) = _prep_edges(edge_index)

    # ---------- layer 1 ----------
    Wc1 = _fold_weights1(W1, att_src1, att_dst1).astype(NPBF16)
    xT = np.zeros((256, NPAD), NPBF16)
    xT[:, :N] = x.T.astype(NPBF16)

    nc1 = _build_layer_program(256, 72, 8, PCH_list, SCH_list, cumS, cumG,
                               layer=1)
    in_maps = [{
        "xT": xT, "wc": Wc1,
        "srcs": np.ascontiguousarray(src_arr[k]),
        "dstf": np.ascontiguousarray(dstf_arr[k]),
        "tbrow": np.ascontiguousarray(tbrow[k]),
        "bias": bias1.astype(np.float32).reshape(1, 64),
    } for k in range(NCORES)]
    kw1 = {}
    if TRACE:
        kw1 = dict(trace=True,
                   tmpdir=(TRACE_DIR + "/l1") if TRACE_DIR else None)
        if kw1["tmpdir"]:
            _os.makedirs(kw1["tmpdir"], exist_ok=True)
    res1 = run_bass_kernel_spmd(nc1, in_maps, core_ids=list(range(NCORES)),
                                **kw1)
    LAST_RESULTS.append(res1)
    # out[k] is [P, TPC, 64], row (p, t) = node (k*TPC+t)*128+p.
    # Assemble x2T [64, NPAD] with node index ((k*TPC+t)*128+p).
    big = np.stack([res1.results[k]["out"] for k in range(NCORES)])
    x2T = np.ascontiguousarray(
        big.transpose(3, 0, 2, 1).reshape(64, NPAD)).astype(NPBF16)

    # ---------- layer 2 ----------
    Wc2 = np.concatenate(
        [W2, W2 @ att_src2.T, W2 @ att_dst2.T], axis=1).astype(NPBF16)

    nc2 = _build_layer_program(64, 65, 1, PCH_list, SCH_list, cumS, cumG,
                               layer=2)
    in_maps2 = [{
        "xT": x2T, "wc": Wc2,
        "srcs": np.ascontiguousarray(src_arr[k]),
        "dstf": np.ascontiguousarray(dstf_arr[k]),
        "tbrow": np.ascontiguousarray(tbrow[k]),
        "bias": bias2.astype(np.float32).reshape(1, 64),
    } for k in range(NCORES)]
    kw2 = {}
    if TRACE:
        kw2 = dict(trace=True,
                   tmpdir=(TRACE_DIR + "/l2") if TRACE_DIR else None)
        if kw2["tmpdir"]:
            _os.makedirs(kw2["tmpdir"], exist_ok=True)
    res2 = run_bass_kernel_spmd(nc2, in_maps2, core_ids=list(range(NCORES)),
                                **kw2)
    LAST_RESULTS.append(res2)
    out = np.stack([res2.results[k]["out"] for k in range(NCORES)])
    out = out.transpose(0, 2, 1, 3).reshape(NPAD, 64)
    return np.ascontiguousarray(out[:N]).astype(np.float32)


# revision 30
# speedup vs baseline: 1.0566x; 1.0309x over previous
"""GAT 2-layer kernel for Trainium2, 8 NeuronCores.

Strategy (graph/data parallel, dst-sharded):
 - Host: sort edges by dst, pack per-core / per-dst-tile chunk streams
   (128 edges per chunk), fold attention vectors into the weight matrix so a
   single matmul produces per-node rows [h | a_src | a_dst].
 - Device, per layer: build T = x @ Wc (node feature table, bf16, in HBM,
   partition-major row order so builds write contiguously at full DMA rate),
   then per dst-tile: per-chunk indirect-DMA gathers of T[src] rows (the HW
   indirect path supports one index per partition per instruction), a_dst
   broadcast to edges via PE (transpose the one-hot S then a small matmul
   against the tile's a_dst rows - no per-edge dst gather), per-edge
   w = exp(leakyrelu(a_src+a_dst)) with leakyrelu as max(x, 0.2x) on DVE so
   the ACT engine only ever holds Exp, and aggregation of numerator +
   denominator with a selection-matrix matmul into PSUM. Outputs are staged
   in SBUF and written once per layer; layer-2 log_softmax defers Ln to a
   single final pass.
 - Two launches (layer1, layer2); host concatenates layer1 shards (the
   "all-to-all halo exchange" of the sharding hint).
"""

import numpy as np
import ml_dtypes
from contextlib import ExitStack

import concourse.bass as bass
import concourse.tile as tile
from concourse import bacc, mybir
from concourse.bass import ts, ds
from concourse.bass_utils import run_bass_kernel_spmd

BF16 = mybir.dt.bfloat16
F32 = mybir.dt.float32
I32 = mybir.dt.int32
NPBF16 = ml_dtypes.bfloat16

P = 128
NCORES = 8
N = 50000
E = 1600000
TPC = 49                      # dst tiles per core
G = NCORES * TPC              # 392 global tiles
NPAD = G * P                  # 50176 padded node count
NEG_SLOPE = 0.2
BG = 28                       # build tiles per group (G = 14*28)
B4 = 4                        # chunks per PSUM batch in the a_dst broadcast

import os as _os
TRACE = bool(_os.environ.get("KERNEL_TRACE"))
TRACE_DIR = _os.environ.get("KERNEL_TRACE_DIR") or None
LAST_RESULTS: list = []
LAST_NCS: list = []


def _prep_edges(edge_index):
    """Sort edges by dst; per (core, tile) split edges into PAIRS (src s and
    s+128 with (s>>7) even - adjacent partition-major table rows, fetched two
    rows per index by one 2D-out indirect DMA) and SINGLES. Streams:
      srcg [P, NG]: one anchor table-row per gather instruction column
        (per tile: PCH pair columns then SCH single columns)
      dstf [P, NS]: dst-local per slot column (per tile: 2*PCH pair slots
        then SCH single slots); -1 pads."""
    src = edge_index[0].astype(np.int64)
    dst = edge_index[1].astype(np.int64)
    # sort by (dst-tile, src) so (tile, src) groups are contiguous
    gt = dst >> 7
    ordr = np.lexsort((src, gt))
    srcs = src[ordr]
    dsts = dst[ordr]
    gte = gt[ordr]

    key = gte * NPAD + srcs
    uk, ust, uc = np.unique(key, return_index=True, return_counts=True)
    ug = uk // NPAD
    us = uk % NPAD
    # partner groups: (g, s) even src-tile -> (g, s+128)
    pk = uk + 128
    pidx = np.searchsorted(uk, pk)
    pidx_c = np.clip(pidx, 0, len(uk) - 1)
    has = (uk[pidx_c] == pk) & (((us >> 7) & 1) == 0) & ((us >> 7) < G - 1)
    npf = np.zeros(len(uk), np.int64)            # pairs where u is FIRST
    npf[has] = np.minimum(uc[has], uc[pidx_c[has]])
    nps = np.zeros(len(uk), np.int64)            # pairs where u is SECOND
    nps[pidx_c[has]] = npf[has]
    # pass B: odd->even tile pairs on leftover edges (no chains: pass A
    # consumed ranks [0, npf+nps) of each group; B takes the next window)
    left = uc - npf - nps
    hasB = (uk[pidx_c] == pk) & (((us >> 7) & 1) == 1) & ((us >> 7) < G - 1)
    npf2 = np.zeros(len(uk), np.int64)
    npf2[hasB] = np.minimum(left[hasB], left[pidx_c[hasB]])
    nps2 = np.zeros(len(uk), np.int64)
    nps2[pidx_c[hasB]] = npf2[hasB]

    E_ = len(srcs)
    grp = np.repeat(np.arange(len(uk)), uc)
    rank_in_grp = np.arange(E_) - ust[grp]
    is_first = rank_in_grp < npf[grp]
    is_second = (~is_first) & (rank_in_grp < (npf + nps)[grp])
    usedA = (npf + nps)[grp]
    is_first2 = (rank_in_grp >= usedA) & (rank_in_grp < usedA + npf2[grp])
    is_second2 = ((rank_in_grp >= usedA + npf2[grp])
                  & (rank_in_grp < usedA + (npf2 + nps2)[grp]))
    is_single = ~(is_first | is_second | is_first2 | is_second2)

    # per-tile pair counts and per-core/per-tile singles
    tile_of_u = ug
    pairs_per_gt = np.bincount(
        tile_of_u, weights=(npf + npf2), minlength=G).astype(np.int64)
    cnt_per_gt = np.bincount(gte, minlength=G)
    sing_per_gt = cnt_per_gt - 2 * pairs_per_gt
    pcg = pairs_per_gt.reshape(NCORES, TPC)
    scg = sing_per_gt.reshape(NCORES, TPC)
    PCH = np.maximum((pcg + P - 1) // P, 0).max(axis=0).astype(np.int64)
    SCH = np.maximum((scg + P - 1) // P, 1).max(axis=0).astype(np.int64)
    CHS = 2 * PCH + SCH                     # slot columns per tile
    GCH = PCH + SCH                         # gather columns per tile
    cumS = np.concatenate([[0], np.cumsum(CHS)]).astype(np.int64)
    cumG = np.concatenate([[0], np.cumsum(GCH)]).astype(np.int64)
    NS = int(cumS[-1])
    NG = int(cumG[-1])

    # tile-local pair index q for firsts: offset of group within tile + rank
    poff_u = np.zeros(len(uk), np.int64)
    # cumsum of npf within each tile
    tile_first_u = np.searchsorted(tile_of_u, np.arange(G), side="left")
    valid = tile_first_u < len(uk)

    def tile_local_cumsum(v):
        cs = np.cumsum(v) - v
        base = np.zeros(G, np.int64)
        base[valid] = cs[tile_first_u[valid]]
        return cs - base[tile_of_u]

    poff_u = tile_local_cumsum(npf)
    pairsA_gt = np.bincount(tile_of_u, weights=npf, minlength=G).astype(
        np.int64)
    # pass-B pair ids come after all pass-A pairs of the tile
    poff2_u = pairsA_gt[tile_of_u] + tile_local_cumsum(npf2)
    q_first = poff_u[grp] + rank_in_grp           # valid where is_first
    poff_partner = np.full(len(uk), -1, np.int64)
    poff_partner[pidx_c[has]] = poff_u[has]
    q_second = poff_partner[grp] + rank_in_grp    # valid where is_second
    q_first2 = poff2_u[grp] + (rank_in_grp - (npf + nps)[grp])
    poff2_partner = np.full(len(uk), -1, np.int64)
    poff2_partner[pidx_c[hasB]] = poff2_u[hasB]
    q_second2 = (poff2_partner[grp]
                 + (rank_in_grp - (npf + nps + npf2)[grp]))

    # tile-local single rank
    sing_cum = np.cumsum(is_single) - is_single
    tile_edge_start = np.concatenate([[0], np.cumsum(cnt_per_gt)])
    sbase = np.zeros(E_, np.int64)
    sbase = sing_cum - (sing_cum[tile_edge_start[gte]] -
                        is_single[tile_edge_start[gte]] * 0)
    r_single = sing_cum - sing_cum[tile_edge_start[gte]]

    core = gte // TPC
    lt = gte % TPC
    row_of = (srcs & 127) * G + (srcs >> 7)

    src_arr = np.zeros((NCORES, P, NG), np.int32)
    dstf_arr = np.full((NCORES, P, NS), -1.0, NPBF16)

    # firsts: gather col cumG[lt]+q>>7 anchor; slots (q&127, cumS+2*(q>>7))
    m = is_first
    j = q_first[m] >> 7
    p = q_first[m] & 127
    src_arr[core[m], p, cumG[lt[m]] + j] = row_of[m].astype(np.int32)
    dstf_arr[core[m], p, cumS[lt[m]] + 2 * j] = (dsts[m] & 127).astype(NPBF16)
    # seconds: slot col +1 (no separate gather col)
    m = is_second
    j = q_second[m] >> 7
    p = q_second[m] & 127
    dstf_arr[core[m], p, cumS[lt[m]] + 2 * j + 1] = (
        dsts[m] & 127).astype(NPBF16)
    # pass-B firsts and seconds
    m = is_first2
    j = q_first2[m] >> 7
    p = q_first2[m] & 127
    src_arr[core[m], p, cumG[lt[m]] + j] = row_of[m].astype(np.int32)
    dstf_arr[core[m], p, cumS[lt[m]] + 2 * j] = (dsts[m] & 127).astype(NPBF16)
    m = is_second2
    j = q_second2[m] >> 7
    p = q_second2[m] & 127
    dstf_arr[core[m], p, cumS[lt[m]] + 2 * j + 1] = (
        dsts[m] & 127).astype(NPBF16)
    # singles: gather col cumG+PCH+r>>7; slot col cumS+2*PCH+r>>7
    m = is_single
    j = r_single[m] >> 7
    p = r_single[m] & 127
    src_arr[core[m], p, cumG[lt[m]] + PCH[lt[m]] + j] = row_of[m].astype(
        np.int32)
    dstf_arr[core[m], p, cumS[lt[m]] + 2 * PCH[lt[m]] + j] = (
        dsts[m] & 127).astype(NPBF16)

    tbrow = np.empty((NCORES, P, TPC), np.int32)
    for k in range(NCORES):
        tbrow[k] = (np.arange(P)[:, None] * G
                    + (k * TPC + np.arange(TPC))[None, :])
    return (src_arr, dstf_arr, tbrow, [int(c) for c in PCH],
            [int(c) for c in SCH], cumS, cumG)


def _build_layer_program(KIN, F_G, F_D, PCH_list, SCH_list, cumS, cumG,
                         layer):
    """One SPMD Bass program for one GAT layer.

    KIN: input feature dim (256 / 64); F_G: gathered row width (64+F_D),
    F_D: heads (8 / 1). Layer 1 outputs bf16 elu(...); layer 2 outputs f32
    log_softmax rows. Output layout [P, TPC, 64]: row (p, t) = node t*128+p.
    """
    F_H = 64
    WCW = F_G + F_D          # built table row width (h | a_src | a_dst)
    RW = F_H + F_D           # matmul rhs / psum width (msg | w)
    KT = (KIN + P - 1) // P  # K tiles for the build matmul
    KP = min(KIN, P)         # partition size of build lhsT
    NS = int(cumS[-1])
    NG = int(cumG[-1])
    CHmax = max(2 * p + s for p, s in zip(PCH_list, SCH_list))
    out_dt = BF16 if layer == 1 else F32
    CW = F_H // F_D

    nc = bacc.Bacc("TRN2", target_bir_lowering=False, debug=False,
                   num_devices=NCORES)

    xT_in = nc.dram_tensor("xT", [KIN, NPAD], BF16, kind="ExternalInput").ap()
    wc_in = nc.dram_tensor("wc", [KIN, WCW], BF16, kind="ExternalInput").ap()
    src_in = nc.dram_tensor("srcs", [P, NG], I32, kind="ExternalInput").ap()
    dstf_in = nc.dram_tensor("dstf", [P, NS], BF16, kind="ExternalInput").ap()
    tbr_in = nc.dram_tensor("tbrow", [P, TPC], I32, kind="ExternalInput").ap()
    bias_in = nc.dram_tensor("bias", [1, F_H], F32, kind="ExternalInput").ap()
    out_dram = nc.dram_tensor("out", [P, TPC, F_H], out_dt,
                              kind="ExternalOutput").ap()

    with tile.TileContext(nc) as tc, ExitStack() as ctx:
        cpool = ctx.enter_context(tc.tile_pool(name="const", bufs=1))
        dpool = ctx.enter_context(tc.tile_pool(name="dram", bufs=1,
                                               space=bass.MemorySpace.DRAM))
        bpool = ctx.enter_context(tc.tile_pool(name="bld", bufs=2))
        stpool = ctx.enter_context(tc.tile_pool(name="bst", bufs=2))
        gpool = ctx.enter_context(tc.tile_pool(name="gat", bufs=2))
        spool = ctx.enter_context(tc.tile_pool(name="sel", bufs=2))
        s2pool = ctx.enter_context(tc.tile_pool(name="s2", bufs=3))
        epool = ctx.enter_context(tc.tile_pool(name="edge", bufs=2))
        opool = ctx.enter_context(tc.tile_pool(name="post", bufs=2))
        tbpool = ctx.enter_context(tc.tile_pool(name="tbt", bufs=1))
        pps = ctx.enter_context(tc.tile_pool(name="psb", bufs=2,
                                             space=bass.MemorySpace.PSUM))
        ppt = ctx.enter_context(tc.tile_pool(name="pst", bufs=2,
                                             space=bass.MemorySpace.PSUM))
        ppa = ctx.enter_context(tc.tile_pool(name="psa", bufs=2,
                                             space=bass.MemorySpace.PSUM))
        ppe = ctx.enter_context(tc.tile_pool(name="pse", bufs=2,
                                             space=bass.MemorySpace.PSUM))

        # ---- constants ----
        wc_sb = cpool.tile([KP, KT, WCW], BF16)
        for kt in range(KT):
            nc.sync.dma_start(wc_sb[:, kt, :], wc_in[kt * KP:(kt + 1) * KP, :])
        bias_sb = cpool.tile([P, F_H], F32)
        nc.sync.dma_start(bias_sb[:], bias_in.to_broadcast((P, F_H)))
        # iota over the d (middle) axis: value = d for all (p, d, c)
        iota_i = stpool.tile([P, P, CHmax], I32)
        nc.gpsimd.iota(iota_i[:], pattern=[[1, P], [0, CHmax]],
                       channel_multiplier=0)
        iota_f = cpool.tile([P, P, CHmax], BF16)
        nc.vector.tensor_copy(iota_f[:], iota_i[:])
        # identity for PE transposes
        idn_p = stpool.tile([P, P], I32)
        nc.gpsimd.iota(idn_p[:], pattern=[[0, P]], channel_multiplier=1)
        idn_f = stpool.tile([P, P], I32)
        nc.gpsimd.iota(idn_f[:], pattern=[[1, P]], channel_multiplier=0)
        ident = cpool.tile([P, P], BF16)
        nc.vector.tensor_tensor(ident[:], idn_p[:], idn_f[:],
                                op=mybir.AluOpType.is_equal)

        # ---- whole-layer edge streams (one DMA each) ----
        src_sb = cpool.tile([P, NG], I32)
        nc.sync.dma_start(src_sb[:], src_in[:])
        dstf_sb = cpool.tile([P, NS], BF16)
        nc.sync.dma_start(dstf_sb[:], dstf_in[:])
        tbr_sb = cpool.tile([P, TPC], I32)
        nc.sync.dma_start(tbr_sb[:], tbr_in[:])

        # ---- output staging ----
        ostage = cpool.tile([P, TPC, F_H], out_dt)
        if layer == 2:
            s_all = cpool.tile([P, TPC], F32)

        # ---- phase 1: build T4 = [h | a_src], Tb4 = [a_dst], both
        # partition-major: node t*128+p at row p*G+t ----
        T4 = dpool.tile([P, G, F_G], BF16)
        Tb4 = dpool.tile([P, G, F_D], BF16)
        T4flat = T4[:, :, :].rearrange("p g f -> (p g) f")
        Tb4flat = Tb4[:, :, :].rearrange("p g f -> (p g) f")
        for gi in range(G // BG):
            xt = bpool.tile([KP, KT, BG * P], BF16)
            for kt in range(KT):
                nc.sync.dma_start(
                    xt[:, kt, :],
                    xT_in[kt * KP:(kt + 1) * KP, gi * BG * P:(gi + 1) * BG * P])
            Tst = stpool.tile([P, BG, F_G], BF16)
            Tbst = stpool.tile([P, BG, F_D], BF16)
            for b0 in range(0, BG, B4):
                nb = min(B4, BG - b0)
                psB = pps.tile([P, B4, WCW], F32)
                for b in range(b0, b0 + nb):
                    for kt in range(KT):
                        nc.tensor.matmul(
                            psB[:, b - b0, :], xt[:, kt, b * P:(b + 1) * P],
                            wc_sb[:, kt, :],
                            start=(kt == 0), stop=(kt == KT - 1))
                nc.vector.tensor_copy(Tst[:, b0:b0 + nb, :],
                                      psB[:, 0:nb, 0:F_G])
                nc.vector.tensor_copy(Tbst[:, b0:b0 + nb, :],
                                      psB[:, 0:nb, F_G:WCW])
            nc.sync.dma_start(T4[:, gi * BG:(gi + 1) * BG, :], Tst[:])
            nc.sync.dma_start(Tb4[:, gi * BG:(gi + 1) * BG, :], Tbst[:])

        # ---- phase 2: per dst-tile edge aggregation ----
        # One run-gather pulls every own tile's a_dst rows: per partition p,
        # Tb4flat rows p*G + k*TPC + t are consecutive for t in [0, TPC), and
        # a 2D-out indirect DMA gathers out-row-width bytes contiguously from
        # each partition's indexed row (HW requires 2D out; 3D outs break).
        tb_all = tbpool.tile([P, TPC * F_D], BF16)
        nc.gpsimd.indirect_dma_start(
            out=tb_all[:], out_offset=None, in_=Tb4flat,
            in_offset=bass.IndirectOffsetOnAxis(
                ap=tbr_sb[:, 0:1], axis=0))
        for t in range(TPC):
            PCH, SCH = PCH_list[t], SCH_list[t]
            CH = 2 * PCH + SCH
            c0 = int(cumS[t])
            g0 = int(cumG[t])
            tb_t = tb_all[:, t * F_D:(t + 1) * F_D]
            # selection one-hot S[e, d, c] = (dst_local[e, c] == d)
            S_t = spool.tile([P, P, CHmax], BF16)
            nc.vector.tensor_tensor(
                S_t[:, :, 0:CH],
                dstf_sb[:, c0:c0 + CH].unsqueeze(1).to_broadcast((P, P, CH)),
                iota_f[:, :, 0:CH], op=mybir.AluOpType.is_equal)
            # gather G rows (HW: one index per partition per instruction;
            # pair columns fetch 2 consecutive table rows into 2 slot cols)
            G_t = gpool.tile([P, CHmax, F_G], BF16)
            G_t2 = G_t[:, :, :].rearrange("p c f -> p (c f)")
            for j in range(PCH):
                nc.gpsimd.indirect_dma_start(
                    out=G_t2[:, 2 * j * F_G:(2 * j + 2) * F_G],
                    out_offset=None, in_=T4flat,
                    in_offset=bass.IndirectOffsetOnAxis(
                        ap=src_sb[:, g0 + j:g0 + j + 1], axis=0))
            for c in range(SCH):
                nc.gpsimd.indirect_dma_start(
                    out=G_t[:, 2 * PCH + c, :], out_offset=None, in_=T4flat,
                    in_offset=bass.IndirectOffsetOnAxis(
                        ap=src_sb[:, g0 + PCH + c:g0 + PCH + c + 1], axis=0))
            # a_dst broadcast to edges: D[e] = (S^T)^T tb = S tb, via PE
            D_t = epool.tile([P, CHmax, F_D], BF16)
            for b0 in range(0, CH, B4):
                n = min(B4, CH - b0)
                psTR = ppt.tile([P, B4, P], BF16)
                for j in range(n):
                    nc.tensor.transpose(psTR[:, j, :], S_t[:, :, b0 + j],
                                        ident[:])
                S2_t = s2pool.tile([P, B4, P], BF16)
                nc.vector.tensor_copy(S2_t[:, 0:n, :], psTR[:, 0:n, :])
                psAD = ppa.tile([P, B4, F_D], F32)
                for j in range(n):
                    nc.tensor.matmul(psAD[:, j, :], S2_t[:, j, :], tb_t,
                                     start=True, stop=True)
                nc.vector.tensor_copy(D_t[:, b0:b0 + n, :], psAD[:, 0:n, :])

            # edge logits -> w = exp(max(s, 0.2s)) (= exp(leakyrelu))
            L_t = epool.tile([P, CHmax, F_D], F32)
            nc.vector.tensor_add(L_t[:, 0:CH, :], G_t[:, 0:CH, F_H:F_G],
                                 D_t[:, 0:CH, :])
            L2_t = epool.tile([P, CHmax, F_D], F32)
            nc.vector.tensor_scalar_mul(L2_t[:, 0:CH, :], L_t[:, 0:CH, :],
                                        NEG_SLOPE)
            nc.vector.tensor_tensor(L2_t[:, 0:CH, :], L_t[:, 0:CH, :],
                                    L2_t[:, 0:CH, :], op=mybir.AluOpType.max)
            rhs_t = epool.tile([P, CHmax, RW], BF16)
            nc.scalar.activation(rhs_t[:, 0:CH, F_H:RW], L2_t[:, 0:CH, :],
                                 mybir.ActivationFunctionType.Exp)
            # msg = w * h[src] (one 4D op; w broadcast across channels)
            nc.vector.tensor_mul(
                rhs_t[:, 0:CH, 0:F_H].rearrange(
                    "p c (h w) -> p c h w", h=F_D),
                G_t[:, 0:CH, 0:F_H].rearrange(
                    "p c (h w) -> p c h w", h=F_D),
                rhs_t[:, 0:CH, F_H:RW].unsqueeze(3).to_broadcast(
                    (P, CH, F_D, CW)))

            psE = ppe.tile([P, RW], F32)
            for c in range(CH):
                nc.tensor.matmul(psE[:], S_t[:, :, c], rhs_t[:, c, :],
                                 start=(c == 0), stop=(c == CH - 1))

            # ---- postprocess this dst-tile ----
            if layer == 1:
                den = opool.tile([P, F_D], F32)
                nc.vector.tensor_scalar_add(den[:], psE[:, F_H:RW], 1e-16)
                rec = opool.tile([P, F_D], F32)
                nc.vector.reciprocal(rec[:], den[:])
                o1 = opool.tile([P, F_H], F32)
                nc.vector.tensor_mul(
                    o1[:].rearrange("p (h w) -> p h w", h=F_D),
                    psE[:, 0:F_H].rearrange("p (h w) -> p h w", h=F_D),
                    rec[:].unsqueeze(2).to_broadcast((P, F_D, CW)))
                nc.vector.tensor_add(o1[:], o1[:], bias_sb[:])
                # elu(x) = max(x,0) + exp(min(x,0)) - 1
                mn = opool.tile([P, F_H], F32)
                nc.vector.tensor_scalar_min(mn[:], o1[:], 0.0)
                em = opool.tile([P, F_H], F32)
                nc.scalar.activation(em[:], mn[:],
                                     mybir.ActivationFunctionType.Exp)
                mx = opool.tile([P, F_H], F32)
                nc.vector.tensor_scalar_max(mx[:], o1[:], 0.0)
                s1 = opool.tile([P, F_H], F32)
                nc.vector.tensor_add(s1[:], mx[:], em[:])
                nc.vector.tensor_scalar_add(ostage[:, t, :], s1[:], -1.0)
            else:
                den = opool.tile([P, 1], F32)
                nc.vector.tensor_scalar_add(den[:], psE[:, F_H:RW], 1e-16)
                rec = opool.tile([P, 1], F32)
                nc.vector.reciprocal(rec[:], den[:])
                o2 = opool.tile([P, F_H], F32)
                nc.vector.tensor_mul(
                    o2[:], psE[:, 0:F_H], rec[:].to_broadcast((P, F_H)))
                nc.vector.tensor_add(o2[:], o2[:], bias_sb[:])
                rm = opool.tile([P, 1], F32)
                nc.vector.tensor_reduce(rm[:], o2[:], mybir.AxisListType.X,
                                        mybir.AluOpType.max)
                nc.vector.tensor_tensor(ostage[:, t, :], o2[:],
                                        rm[:].to_broadcast((P, F_H)),
                                        op=mybir.AluOpType.subtract)
                e_t = opool.tile([P, F_H], F32)
                nc.scalar.activation(e_t[:], ostage[:, t, :],
                                     mybir.ActivationFunctionType.Exp,
                                     accum_out=s_all[:, t:t + 1])

        if layer == 2:
            # deferred log-softmax denominator: one Ln pass + one subtract
            ls = cpool.tile([P, TPC], F32)
            nc.scalar.activation(ls[:], s_all[:],
                                 mybir.ActivationFunctionType.Ln)
            nc.vector.tensor_tensor(
                ostage[:], ostage[:],
                ls[:].unsqueeze(2).to_broadcast((P, TPC, F_H)),
                op=mybir.AluOpType.subtract)
        nc.sync.dma_start(out_dram[:], ostage[:])

    nc.compile()
    LAST_NCS.append(nc)
    return nc


def _fold_weights1(W1, att_src1, att_dst1):
    A1s = np.zeros((64, 8), np.float32)
    A1s[np.arange(64), np.arange(64) // 8] = att_src1.reshape(64)
    A1d = np.zeros((64, 8), np.float32)
    A1d[np.arange(64), np.arange(64) // 8] = att_dst1.reshape(64)
    return np.concatenate([W1, W1 @ A1s, W1 @ A1d], axis=1)  # [256, 80]


def kernel(x, edge_index, W1, att_src1, att_dst1, bias1,
           W2, att_src2, att_dst2, bias2):
    x, edge_index = np.asarray(x), np.asarray(edge_index)
    W1, att_src1 = np.asarray(W1), np.asarray(att_src1)
    att_dst1, bias1 = np.asarray(att_dst1), np.asarray(bias1)
    W2, att_src2 = np.asarray(W2), np.asarray(att_src2)
    att_dst2, bias2 = np.asarray(att_dst2), np.asarray(bias2)
    LAST_RESULTS.clear()
    LAST_NCS.clear()
    (src_arr, dstf_arr, tbrow, PCH_list, SCH_list,
     cumS, cumG) = _prep_edges(edge_index)

    # ---------- layer 1 ----------
    Wc1 = _fold_weights1(W1, att_src1, att_dst1).astype(NPBF16)
    xT = np.zeros((256, NPAD), NPBF16)
    xT[:, :N] = x.T.astype(NPBF16)

    nc1 = _build_layer_program(256, 72, 8, PCH_list, SCH_list, cumS, cumG,
                               layer=1)
    in_maps = [{
        "xT": xT, "wc": Wc1,
        "srcs": np.ascontiguousarray(src_arr[k]),
        "dstf": np.ascontiguousarray(dstf_arr[k]),
        "tbrow": np.ascontiguousarray(tbrow[k]),
        "bias": bias1.astype(np.float32).reshape(1, 64),
    } for k in range(NCORES)]
    kw1 = {}
    if TRACE:
        kw1 = dict(trace=True,
                   tmpdir=(TRACE_DIR + "/l1") if TRACE_DIR else None)
        if kw1["tmpdir"]:
            _os.makedirs(kw1["tmpdir"], exist_ok=True)
    res1 = run_bass_kernel_spmd(nc1, in_maps, core_ids=list(range(NCORES)),
                                **kw1)
    LAST_RESULTS.append(res1)
    # out[k] is [P, TPC, 64], row (p, t) = node (k*TPC+t)*128+p.
    # Assemble x2T [64, NPAD] with node index ((k*TPC+t)*128+p).
    big = np.stack([res1.results[k]["out"] for k in range(NCORES)])
    x2T = np.ascontiguousarray(
        big.transpose(3, 0, 2, 1).reshape(64, NPAD)).astype(NPBF16)

    # ---------- layer 2 ----------
    Wc2 = np.concatenate(
        [W2, W2 @ att_src2.T, W2 @ att_dst2.T], axis=1).astype(NPBF16)

    nc2 = _build_layer_program(64, 65, 1, PCH_list, SCH_list, cumS, cumG,
                               layer=2)
    in_maps2 = [{
        "xT": x2T, "wc": Wc2,
        "srcs": np.ascontiguousarray(src_arr[k]),
        "dstf": np.ascontiguousarray(dstf_arr[k]),
        "tbrow": np.ascontiguousarray(tbrow[k]),
        "bias": bias2.astype(np.float32).reshape(1, 64),
    } for k in range(NCORES)]
    kw2 = {}
    if TRACE:
        kw2 = dict(trace=True,
                   tmpdir=(TRACE_DIR + "/l2") if TRACE_DIR else None)
        if kw2["tmpdir"]:
            _os.makedirs(kw2["tmpdir"], exist_ok=True)
    res2 = run_bass_kernel_spmd(nc2, in_maps2, core_ids=list(range(NCORES)),
                                **kw2)
    LAST_RESULTS.append(res2)
    out = np.stack([res2.results[k]["out"] for k in range(NCORES)])
    out = out.transpose(0, 2, 1, 3).reshape(NPAD, 64)
    return np.ascontiguousarray(out[:N]).astype(np.float32)


# revision 31
# speedup vs baseline: 1.0811x; 1.0231x over previous
"""GAT 2-layer kernel for Trainium2, 8 NeuronCores.

Strategy (graph/data parallel, dst-sharded):
 - Host: sort edges by dst, pack per-core / per-dst-tile chunk streams
   (128 edges per chunk), fold attention vectors into the weight matrix so a
   single matmul produces per-node rows [h | a_src | a_dst].
 - Device, per layer: build T = x @ Wc (node feature table, bf16, in HBM,
   partition-major row order so builds write contiguously at full DMA rate),
   then per dst-tile: per-chunk indirect-DMA gathers of T[src] rows (the HW
   indirect path supports one index per partition per instruction), a_dst
   broadcast to edges via PE (transpose the one-hot S then a small matmul
   against the tile's a_dst rows - no per-edge dst gather), per-edge
   w = exp(leakyrelu(a_src+a_dst)) with leakyrelu as max(x, 0.2x) on DVE so
   the ACT engine only ever holds Exp, and aggregation of numerator +
   denominator with a selection-matrix matmul into PSUM. Outputs are staged
   in SBUF and written once per layer; layer-2 log_softmax defers Ln to a
   single final pass.
 - Two launches (layer1, layer2); host concatenates layer1 shards (the
   "all-to-all halo exchange" of the sharding hint).
"""

import numpy as np
import ml_dtypes
from contextlib import ExitStack

import concourse.bass as bass
import concourse.tile as tile
from concourse import bacc, mybir
from concourse.bass import ts, ds
from concourse.bass_utils import run_bass_kernel_spmd

BF16 = mybir.dt.bfloat16
F32 = mybir.dt.float32
I32 = mybir.dt.int32
NPBF16 = ml_dtypes.bfloat16

P = 128
NCORES = 8
N = 50000
E = 1600000
TPC = 49                      # dst tiles per core
G = NCORES * TPC              # 392 global tiles
NPAD = G * P                  # 50176 padded node count
NEG_SLOPE = 0.2
BG = 28                       # build tiles per group (G = 14*28)
B4 = 4                        # chunks per PSUM batch in the a_dst broadcast

import os as _os
TRACE = bool(_os.environ.get("KERNEL_TRACE"))
TRACE_DIR = _os.environ.get("KERNEL_TRACE_DIR") or None
LAST_RESULTS: list = []
LAST_NCS: list = []


def _prep_edges(edge_index):
    """Sort edges by dst; per (core, tile) split edges into PAIRS (src s and
    s+128 with (s>>7) even - adjacent partition-major table rows, fetched two
    rows per index by one 2D-out indirect DMA) and SINGLES. Streams:
      srcg [P, NG]: one anchor table-row per gather instruction column
        (per tile: PCH pair columns then SCH single columns)
      dstf [P, NS]: dst-local per slot column (per tile: 2*PCH pair slots
        then SCH single slots); -1 pads."""
    src = edge_index[0].astype(np.int64)
    dst = edge_index[1].astype(np.int64)
    # sort by (dst-tile, src) so (tile, src) groups are contiguous
    gt = dst >> 7
    ordr = np.lexsort((src, gt))
    srcs = src[ordr]
    dsts = dst[ordr]
    gte = gt[ordr]

    key = gte * NPAD + srcs
    uk, ust, uc = np.unique(key, return_index=True, return_counts=True)
    ug = uk // NPAD
    us = uk % NPAD
    # partner groups: (g, s) even src-tile -> (g, s+128)
    pk = uk + 128
    pidx = np.searchsorted(uk, pk)
    pidx_c = np.clip(pidx, 0, len(uk) - 1)
    has = (uk[pidx_c] == pk) & (((us >> 7) & 1) == 0) & ((us >> 7) < G - 1)
    npf = np.zeros(len(uk), np.int64)            # pairs where u is FIRST
    npf[has] = np.minimum(uc[has], uc[pidx_c[has]])
    nps = np.zeros(len(uk), np.int64)            # pairs where u is SECOND
    nps[pidx_c[has]] = npf[has]
    # pass B: odd->even tile pairs on leftover edges (no chains: pass A
    # consumed ranks [0, npf+nps) of each group; B takes the next window)
    left = uc - npf - nps
    hasB = (uk[pidx_c] == pk) & (((us >> 7) & 1) == 1) & ((us >> 7) < G - 1)
    npf2 = np.zeros(len(uk), np.int64)
    npf2[hasB] = np.minimum(left[hasB], left[pidx_c[hasB]])
    nps2 = np.zeros(len(uk), np.int64)
    nps2[pidx_c[hasB]] = npf2[hasB]

    E_ = len(srcs)
    grp = np.repeat(np.arange(len(uk)), uc)
    rank_in_grp = np.arange(E_) - ust[grp]
    is_first = rank_in_grp < npf[grp]
    is_second = (~is_first) & (rank_in_grp < (npf + nps)[grp])
    usedA = (npf + nps)[grp]
    is_first2 = (rank_in_grp >= usedA) & (rank_in_grp < usedA + npf2[grp])
    is_second2 = ((rank_in_grp >= usedA + npf2[grp])
                  & (rank_in_grp < usedA + (npf2 + nps2)[grp]))
    is_single = ~(is_first | is_second | is_first2 | is_second2)

    # per-tile pair counts and per-core/per-tile singles
    tile_of_u = ug
    pairs_per_gt = np.bincount(
        tile_of_u, weights=(npf + npf2), minlength=G).astype(np.int64)
    cnt_per_gt = np.bincount(gte, minlength=G)
    sing_per_gt = cnt_per_gt - 2 * pairs_per_gt
    pcg = pairs_per_gt.reshape(NCORES, TPC)
    scg = sing_per_gt.reshape(NCORES, TPC)
    PCH = np.maximum((pcg + P - 1) // P, 0).max(axis=0).astype(np.int64)
    # padded anchor positions in the pair region carry singles (their
    # second fetched row is dstf=-1 masked), shrinking the single region
    pad_kt = PCH[None, :] * P - pcg
    rem_kt = np.maximum(scg - pad_kt, 0)
    SCH = np.maximum((rem_kt + P - 1) // P, 1).max(axis=0).astype(np.int64)
    CHS = 2 * PCH + SCH                     # slot columns per tile
    GCH = PCH + SCH                         # gather columns per tile
    cumS = np.concatenate([[0], np.cumsum(CHS)]).astype(np.int64)
    cumG = np.concatenate([[0], np.cumsum(GCH)]).astype(np.int64)
    NS = int(cumS[-1])
    NG = int(cumG[-1])

    # tile-local pair index q for firsts: offset of group within tile + rank
    poff_u = np.zeros(len(uk), np.int64)
    # cumsum of npf within each tile
    tile_first_u = np.searchsorted(tile_of_u, np.arange(G), side="left")
    valid = tile_first_u < len(uk)

    def tile_local_cumsum(v):
        cs = np.cumsum(v) - v
        base = np.zeros(G, np.int64)
        base[valid] = cs[tile_first_u[valid]]
        return cs - base[tile_of_u]

    poff_u = tile_local_cumsum(npf)
    pairsA_gt = np.bincount(tile_of_u, weights=npf, minlength=G).astype(
        np.int64)
    # pass-B pair ids come after all pass-A pairs of the tile
    poff2_u = pairsA_gt[tile_of_u] + tile_local_cumsum(npf2)
    q_first = poff_u[grp] + rank_in_grp           # valid where is_first
    poff_partner = np.full(len(uk), -1, np.int64)
    poff_partner[pidx_c[has]] = poff_u[has]
    q_second = poff_partner[grp] + rank_in_grp    # valid where is_second
    q_first2 = poff2_u[grp] + (rank_in_grp - (npf + nps)[grp])
    poff2_partner = np.full(len(uk), -1, np.int64)
    poff2_partner[pidx_c[hasB]] = poff2_u[hasB]
    q_second2 = (poff2_partner[grp]
                 + (rank_in_grp - (npf + nps + npf2)[grp]))

    # tile-local single rank
    sing_cum = np.cumsum(is_single) - is_single
    tile_edge_start = np.concatenate([[0], np.cumsum(cnt_per_gt)])
    sbase = np.zeros(E_, np.int64)
    sbase = sing_cum - (sing_cum[tile_edge_start[gte]] -
                        is_single[tile_edge_start[gte]] * 0)
    r_single = sing_cum - sing_cum[tile_edge_start[gte]]

    core = gte // TPC
    lt = gte % TPC
    row_of = (srcs & 127) * G + (srcs >> 7)

    src_arr = np.zeros((NCORES, P, NG), np.int32)
    dstf_arr = np.full((NCORES, P, NS), -1.0, NPBF16)

    # firsts: gather col cumG[lt]+q>>7 anchor; slots (q&127, cumS+2*(q>>7))
    m = is_first
    j = q_first[m] >> 7
    p = q_first[m] & 127
    src_arr[core[m], p, cumG[lt[m]] + j] = row_of[m].astype(np.int32)
    dstf_arr[core[m], p, cumS[lt[m]] + 2 * j] = (dsts[m] & 127).astype(NPBF16)
    # seconds: slot col +1 (no separate gather col)
    m = is_second
    j = q_second[m] >> 7
    p = q_second[m] & 127
    dstf_arr[core[m], p, cumS[lt[m]] + 2 * j + 1] = (
        dsts[m] & 127).astype(NPBF16)
    # pass-B firsts and seconds
    m = is_first2
    j = q_first2[m] >> 7
    p = q_first2[m] & 127
    src_arr[core[m], p, cumG[lt[m]] + j] = row_of[m].astype(np.int32)
    dstf_arr[core[m], p, cumS[lt[m]] + 2 * j] = (dsts[m] & 127).astype(NPBF16)
    m = is_second2
    j = q_second2[m] >> 7
    p = q_second2[m] & 127
    dstf_arr[core[m], p, cumS[lt[m]] + 2 * j + 1] = (
        dsts[m] & 127).astype(NPBF16)
    # singles: first pad_g of them ride padded pair anchors (1 useful
    # slot each); the rest go to the single region
    pad_e = (PCH[lt] * P - pairs_per_gt[gte])
    m = is_single & (r_single < pad_e)
    q = pairs_per_gt[gte[m]] + r_single[m]
    j = q >> 7
    p = q & 127
    src_arr[core[m], p, cumG[lt[m]] + j] = row_of[m].astype(np.int32)
    dstf_arr[core[m], p, cumS[lt[m]] + 2 * j] = (dsts[m] & 127).astype(NPBF16)
    m = is_single & (r_single >= pad_e)
    r2 = r_single[m] - pad_e[m]
    j = r2 >> 7
    p = r2 & 127
    src_arr[core[m], p, cumG[lt[m]] + PCH[lt[m]] + j] = row_of[m].astype(
        np.int32)
    dstf_arr[core[m], p, cumS[lt[m]] + 2 * PCH[lt[m]] + j] = (
        dsts[m] & 127).astype(NPBF16)

    tbrow = np.empty((NCORES, P, TPC), np.int32)
    for k in range(NCORES):
        tbrow[k] = (np.arange(P)[:, None] * G
                    + (k * TPC + np.arange(TPC))[None, :])
    return (src_arr, dstf_arr, tbrow, [int(c) for c in PCH],
            [int(c) for c in SCH], cumS, cumG)


def _build_layer_program(KIN, F_G, F_D, PCH_list, SCH_list, cumS, cumG,
                         layer):
    """One SPMD Bass program for one GAT layer.

    KIN: input feature dim (256 / 64); F_G: gathered row width (64+F_D),
    F_D: heads (8 / 1). Layer 1 outputs bf16 elu(...); layer 2 outputs f32
    log_softmax rows. Output layout [P, TPC, 64]: row (p, t) = node t*128+p.
    """
    F_H = 64
    WCW = F_G + F_D          # built table row width (h | a_src | a_dst)
    RW = F_H + F_D           # matmul rhs / psum width (msg | w)
    KT = (KIN + P - 1) // P  # K tiles for the build matmul
    KP = min(KIN, P)         # partition size of build lhsT
    NS = int(cumS[-1])
    NG = int(cumG[-1])
    CHmax = max(2 * p + s for p, s in zip(PCH_list, SCH_list))
    out_dt = BF16 if layer == 1 else F32
    CW = F_H // F_D

    nc = bacc.Bacc("TRN2", target_bir_lowering=False, debug=False,
                   num_devices=NCORES)

    xT_in = nc.dram_tensor("xT", [KIN, NPAD], BF16, kind="ExternalInput").ap()
    wc_in = nc.dram_tensor("wc", [KIN, WCW], BF16, kind="ExternalInput").ap()
    src_in = nc.dram_tensor("srcs", [P, NG], I32, kind="ExternalInput").ap()
    dstf_in = nc.dram_tensor("dstf", [P, NS], BF16, kind="ExternalInput").ap()
    tbr_in = nc.dram_tensor("tbrow", [P, TPC], I32, kind="ExternalInput").ap()
    bias_in = nc.dram_tensor("bias", [1, F_H], F32, kind="ExternalInput").ap()
    out_dram = nc.dram_tensor("out", [P, TPC, F_H], out_dt,
                              kind="ExternalOutput").ap()

    with tile.TileContext(nc) as tc, ExitStack() as ctx:
        cpool = ctx.enter_context(tc.tile_pool(name="const", bufs=1))
        dpool = ctx.enter_context(tc.tile_pool(name="dram", bufs=1,
                                               space=bass.MemorySpace.DRAM))
        bpool = ctx.enter_context(tc.tile_pool(name="bld", bufs=2))
        stpool = ctx.enter_context(tc.tile_pool(name="bst", bufs=2))
        gpool = ctx.enter_context(tc.tile_pool(name="gat", bufs=2))
        spool = ctx.enter_context(tc.tile_pool(name="sel", bufs=2))
        s2pool = ctx.enter_context(tc.tile_pool(name="s2", bufs=3))
        epool = ctx.enter_context(tc.tile_pool(name="edge", bufs=2))
        opool = ctx.enter_context(tc.tile_pool(name="post", bufs=2))
        tbpool = ctx.enter_context(tc.tile_pool(name="tbt", bufs=1))
        pps = ctx.enter_context(tc.tile_pool(name="psb", bufs=2,
                                             space=bass.MemorySpace.PSUM))
        ppt = ctx.enter_context(tc.tile_pool(name="pst", bufs=2,
                                             space=bass.MemorySpace.PSUM))
        ppa = ctx.enter_context(tc.tile_pool(name="psa", bufs=2,
                                             space=bass.MemorySpace.PSUM))
        ppe = ctx.enter_context(tc.tile_pool(name="pse", bufs=2,
                                             space=bass.MemorySpace.PSUM))

        # ---- constants ----
        wc_sb = cpool.tile([KP, KT, WCW], BF16)
        for kt in range(KT):
            nc.sync.dma_start(wc_sb[:, kt, :], wc_in[kt * KP:(kt + 1) * KP, :])
        bias_sb = cpool.tile([P, F_H], F32)
        nc.sync.dma_start(bias_sb[:], bias_in.to_broadcast((P, F_H)))
        # iota over the d (middle) axis: value = d for all (p, d, c)
        iota_i = stpool.tile([P, P, CHmax], I32)
        nc.gpsimd.iota(iota_i[:], pattern=[[1, P], [0, CHmax]],
                       channel_multiplier=0)
        iota_f = cpool.tile([P, P, CHmax], BF16)
        nc.vector.tensor_copy(iota_f[:], iota_i[:])
        # identity for PE transposes
        idn_p = stpool.tile([P, P], I32)
        nc.gpsimd.iota(idn_p[:], pattern=[[0, P]], channel_multiplier=1)
        idn_f = stpool.tile([P, P], I32)
        nc.gpsimd.iota(idn_f[:], pattern=[[1, P]], channel_multiplier=0)
        ident = cpool.tile([P, P], BF16)
        nc.vector.tensor_tensor(ident[:], idn_p[:], idn_f[:],
                                op=mybir.AluOpType.is_equal)

        # ---- whole-layer edge streams (one DMA each) ----
        src_sb = cpool.tile([P, NG], I32)
        nc.sync.dma_start(src_sb[:], src_in[:])
        dstf_sb = cpool.tile([P, NS], BF16)
        nc.sync.dma_start(dstf_sb[:], dstf_in[:])
        tbr_sb = cpool.tile([P, TPC], I32)
        nc.sync.dma_start(tbr_sb[:], tbr_in[:])

        # ---- output staging ----
        ostage = cpool.tile([P, TPC, F_H], out_dt)
        if layer == 2:
            s_all = cpool.tile([P, TPC], F32)

        # ---- phase 1: build T4 = [h | a_src], Tb4 = [a_dst], both
        # partition-major: node t*128+p at row p*G+t ----
        T4 = dpool.tile([P, G, F_G], BF16)
        Tb4 = dpool.tile([P, G, F_D], BF16)
        T4flat = T4[:, :, :].rearrange("p g f -> (p g) f")
        Tb4flat = Tb4[:, :, :].rearrange("p g f -> (p g) f")
        for gi in range(G // BG):
            xt = bpool.tile([KP, KT, BG * P], BF16)
            for kt in range(KT):
                nc.sync.dma_start(
                    xt[:, kt, :],
                    xT_in[kt * KP:(kt + 1) * KP, gi * BG * P:(gi + 1) * BG * P])
            Tst = stpool.tile([P, BG, F_G], BF16)
            Tbst = stpool.tile([P, BG, F_D], BF16)
            for b0 in range(0, BG, B4):
                nb = min(B4, BG - b0)
                psB = pps.tile([P, B4, WCW], F32)
                for b in range(b0, b0 + nb):
                    for kt in range(KT):
                        nc.tensor.matmul(
                            psB[:, b - b0, :], xt[:, kt, b * P:(b + 1) * P],
                            wc_sb[:, kt, :],
                            start=(kt == 0), stop=(kt == KT - 1))
                nc.vector.tensor_copy(Tst[:, b0:b0 + nb, :],
                                      psB[:, 0:nb, 0:F_G])
                nc.vector.tensor_copy(Tbst[:, b0:b0 + nb, :],
                                      psB[:, 0:nb, F_G:WCW])
            nc.sync.dma_start(T4[:, gi * BG:(gi + 1) * BG, :], Tst[:])
            nc.sync.dma_start(Tb4[:, gi * BG:(gi + 1) * BG, :], Tbst[:])

        # ---- phase 2: per dst-tile edge aggregation ----
        # One run-gather pulls every own tile's a_dst rows: per partition p,
        # Tb4flat rows p*G + k*TPC + t are consecutive for t in [0, TPC), and
        # a 2D-out indirect DMA gathers out-row-width bytes contiguously from
        # each partition's indexed row (HW requires 2D out; 3D outs break).
        tb_all = tbpool.tile([P, TPC * F_D], BF16)
        nc.gpsimd.indirect_dma_start(
            out=tb_all[:], out_offset=None, in_=Tb4flat,
            in_offset=bass.IndirectOffsetOnAxis(
                ap=tbr_sb[:, 0:1], axis=0))
        for t in range(TPC):
            PCH, SCH = PCH_list[t], SCH_list[t]
            CH = 2 * PCH + SCH
            c0 = int(cumS[t])
            g0 = int(cumG[t])
            tb_t = tb_all[:, t * F_D:(t + 1) * F_D]
            # selection one-hot S[e, d, c] = (dst_local[e, c] == d)
            S_t = spool.tile([P, P, CHmax], BF16)
            nc.vector.tensor_tensor(
                S_t[:, :, 0:CH],
                dstf_sb[:, c0:c0 + CH].unsqueeze(1).to_broadcast((P, P, CH)),
                iota_f[:, :, 0:CH], op=mybir.AluOpType.is_equal)
            # gather G rows (HW: one index per partition per instruction;
            # pair columns fetch 2 consecutive table rows into 2 slot cols)
            G_t = gpool.tile([P, CHmax, F_G], BF16)
            G_t2 = G_t[:, :, :].rearrange("p c f -> p (c f)")
            for j in range(PCH):
                nc.gpsimd.indirect_dma_start(
                    out=G_t2[:, 2 * j * F_G:(2 * j + 2) * F_G],
                    out_offset=None, in_=T4flat,
                    in_offset=bass.IndirectOffsetOnAxis(
                        ap=src_sb[:, g0 + j:g0 + j + 1], axis=0))
            for c in range(SCH):
                nc.gpsimd.indirect_dma_start(
                    out=G_t[:, 2 * PCH + c, :], out_offset=None, in_=T4flat,
                    in_offset=bass.IndirectOffsetOnAxis(
                        ap=src_sb[:, g0 + PCH + c:g0 + PCH + c + 1], axis=0))
            # a_dst broadcast to edges: D[e] = (S^T)^T tb = S tb, via PE
            D_t = epool.tile([P, CHmax, F_D], BF16)
            for b0 in range(0, CH, B4):
                n = min(B4, CH - b0)
                psTR = ppt.tile([P, B4, P], BF16)
                for j in range(n):
                    nc.tensor.transpose(psTR[:, j, :], S_t[:, :, b0 + j],
                                        ident[:])
                S2_t = s2pool.tile([P, B4, P], BF16)
                nc.vector.tensor_copy(S2_t[:, 0:n, :], psTR[:, 0:n, :])
                psAD = ppa.tile([P, B4, F_D], F32)
                for j in range(n):
                    nc.tensor.matmul(psAD[:, j, :], S2_t[:, j, :], tb_t,
                                     start=True, stop=True)
                nc.vector.tensor_copy(D_t[:, b0:b0 + n, :], psAD[:, 0:n, :])

            # edge logits -> w = exp(max(s, 0.2s)) (= exp(leakyrelu))
            L_t = epool.tile([P, CHmax, F_D], F32)
            nc.vector.tensor_add(L_t[:, 0:CH, :], G_t[:, 0:CH, F_H:F_G],
                                 D_t[:, 0:CH, :])
            L2_t = epool.tile([P, CHmax, F_D], F32)
            nc.vector.tensor_scalar_mul(L2_t[:, 0:CH, :], L_t[:, 0:CH, :],
                                        NEG_SLOPE)
            nc.vector.tensor_tensor(L2_t[:, 0:CH, :], L_t[:, 0:CH, :],
                                    L2_t[:, 0:CH, :], op=mybir.AluOpType.max)
            rhs_t = epool.tile([P, CHmax, RW], BF16)
            nc.scalar.activation(rhs_t[:, 0:CH, F_H:RW], L2_t[:, 0:CH, :],
                                 mybir.ActivationFunctionType.Exp)
            # msg = w * h[src] (one 4D op; w broadcast across channels)
            nc.vector.tensor_mul(
                rhs_t[:, 0:CH, 0:F_H].rearrange(
                    "p c (h w) -> p c h w", h=F_D),
                G_t[:, 0:CH, 0:F_H].rearrange(
                    "p c (h w) -> p c h w", h=F_D),
                rhs_t[:, 0:CH, F_H:RW].unsqueeze(3).to_broadcast(
                    (P, CH, F_D, CW)))

            psE = ppe.tile([P, RW], F32)
            for c in range(CH):
                nc.tensor.matmul(psE[:], S_t[:, :, c], rhs_t[:, c, :],
                                 start=(c == 0), stop=(c == CH - 1))

            # ---- postprocess this dst-tile ----
            if layer == 1:
                den = opool.tile([P, F_D], F32)
                nc.vector.tensor_scalar_add(den[:], psE[:, F_H:RW], 1e-16)
                rec = opool.tile([P, F_D], F32)
                nc.vector.reciprocal(rec[:], den[:])
                o1 = opool.tile([P, F_H], F32)
                nc.vector.tensor_mul(
                    o1[:].rearrange("p (h w) -> p h w", h=F_D),
                    psE[:, 0:F_H].rearrange("p (h w) -> p h w", h=F_D),
                    rec[:].unsqueeze(2).to_broadcast((P, F_D, CW)))
                nc.vector.tensor_add(o1[:], o1[:], bias_sb[:])
                # elu(x) = max(x,0) + exp(min(x,0)) - 1
                mn = opool.tile([P, F_H], F32)
                nc.vector.tensor_scalar_min(mn[:], o1[:], 0.0)
                em = opool.tile([P, F_H], F32)
                nc.scalar.activation(em[:], mn[:],
                                     mybir.ActivationFunctionType.Exp)
                mx = opool.tile([P, F_H], F32)
                nc.vector.tensor_scalar_max(mx[:], o1[:], 0.0)
                s1 = opool.tile([P, F_H], F32)
                nc.vector.tensor_add(s1[:], mx[:], em[:])
                nc.vector.tensor_scalar_add(ostage[:, t, :], s1[:], -1.0)
            else:
                den = opool.tile([P, 1], F32)
                nc.vector.tensor_scalar_add(den[:], psE[:, F_H:RW], 1e-16)
                rec = opool.tile([P, 1], F32)
                nc.vector.reciprocal(rec[:], den[:])
                o2 = opool.tile([P, F_H], F32)
                nc.vector.tensor_mul(
                    o2[:], psE[:, 0:F_H], rec[:].to_broadcast((P, F_H)))
                nc.vector.tensor_add(o2[:], o2[:], bias_sb[:])
                rm = opool.tile([P, 1], F32)
                nc.vector.tensor_reduce(rm[:], o2[:], mybir.AxisListType.X,
                                        mybir.AluOpType.max)
                nc.vector.tensor_tensor(ostage[:, t, :], o2[:],
                                        rm[:].to_broadcast((P, F_H)),
                                        op=mybir.AluOpType.subtract)
                e_t = opool.tile([P, F_H], F32)
                nc.scalar.activation(e_t[:], ostage[:, t, :],
                                     mybir.ActivationFunctionType.Exp,
                                     accum_out=s_all[:, t:t + 1])

        if layer == 2:
            # deferred log-softmax denominator: one Ln pass + one subtract
            ls = cpool.tile([P, TPC], F32)
            nc.scalar.activation(ls[:], s_all[:],
                                 mybir.ActivationFunctionType.Ln)
            nc.vector.tensor_tensor(
                ostage[:], ostage[:],
                ls[:].unsqueeze(2).to_broadcast((P, TPC, F_H)),
                op=mybir.AluOpType.subtract)
        nc.sync.dma_start(out_dram[:], ostage[:])

    nc.compile()
    LAST_NCS.append(nc)
    return nc


def _fold_weights1(W1, att_src1, att_dst1):
    A1s = np.zeros((64, 8), np.float32)
    A1s[np.arange(64), np.arange(64) // 8] = att_src1.reshape(64)
    A1d = np.zeros((64, 8), np.float32)
    A1d[np.arange(64), np.arange(64) // 8] = att_dst1.reshape(64)
    return np.concatenate([W1, W1 @ A1s, W1 @ A1d], axis=1)  # [256, 80]


def kernel(x, edge_index, W1, att_src1, att_dst1, bias1,
           W2, att_src2, att_dst2, bias2):
    x, edge_index = np.asarray(x), np.asarray(edge_index)
    W1, att_src1 = np.asarray(W1), np.asarray(att_src1)
    att_dst1, bias1 = np.asarray(att_dst1), np.asarray(bias1)
    W2, att_src2 = np.asarray(W2), np.asarray(att_src2)
    att_dst2, bias2 = np.asarray(att_dst2), np.asarray(bias2)
    LAST_RESULTS.clear()
    LAST_NCS.clear()
    (src_arr, dstf_arr, tbrow, PCH_list, SCH_list,
     cumS, cumG) = _prep_edges(edge_index)

    # ---------- layer 1 ----------
    Wc1 = _fold_weights1(W1, att_src1, att_dst1).astype(NPBF16)
    xT = np.zeros((256, NPAD), NPBF16)
    xT[:, :N] = x.T.astype(NPBF16)

    nc1 = _build_layer_program(256, 72, 8, PCH_list, SCH_list, cumS, cumG,
                               layer=1)
    in_maps = [{
        "xT": xT, "wc": Wc1,
        "srcs": np.ascontiguousarray(src_arr[k]),
        "dstf": np.ascontiguousarray(dstf_arr[k]),
        "tbrow": np.ascontiguousarray(tbrow[k]),
        "bias": bias1.astype(np.float32).reshape(1, 64),
    } for k in range(NCORES)]
    kw1 = {}
    if TRACE:
        kw1 = dict(trace=True,
                   tmpdir=(TRACE_DIR + "/l1") if TRACE_DIR else None)
        if kw1["tmpdir"]:
            _os.makedirs(kw1["tmpdir"], exist_ok=True)
    res1 = run_bass_kernel_spmd(nc1, in_maps, core_ids=list(range(NCORES)),
                                **kw1)
    LAST_RESULTS.append(res1)
    # out[k] is [P, TPC, 64], row (p, t) = node (k*TPC+t)*128+p.
    # Assemble x2T [64, NPAD] with node index ((k*TPC+t)*128+p).
    big = np.stack([res1.results[k]["out"] for k in range(NCORES)])
    x2T = np.ascontiguousarray(
        big.transpose(3, 0, 2, 1).reshape(64, NPAD)).astype(NPBF16)

    # ---------- layer 2 ----------
    Wc2 = np.concatenate(
        [W2, W2 @ att_src2.T, W2 @ att_dst2.T], axis=1).astype(NPBF16)

    nc2 = _build_layer_program(64, 65, 1, PCH_list, SCH_list, cumS, cumG,
                               layer=2)
    in_maps2 = [{
        "xT": x2T, "wc": Wc2,
        "srcs": np.ascontiguousarray(src_arr[k]),
        "dstf": np.ascontiguousarray(dstf_arr[k]),
        "tbrow": np.ascontiguousarray(tbrow[k]),
        "bias": bias2.astype(np.float32).reshape(1, 64),
    } for k in range(NCORES)]
    kw2 = {}
    if TRACE:
        kw2 = dict(trace=True,
                   tmpdir=(TRACE_DIR + "/l2") if TRACE_DIR else None)
        if kw2["tmpdir"]:
            _os.makedirs(kw2["tmpdir"], exist_ok=True)
    res2 = run_bass_kernel_spmd(nc2, in_maps2, core_ids=list(range(NCORES)),
                                **kw2)
    LAST_RESULTS.append(res2)
    out = np.stack([res2.results[k]["out"] for k in range(NCORES)])
    out = out.transpose(0, 2, 1, 3).reshape(NPAD, 64)
    return np.ascontiguousarray(out[:N]).astype(np.float32)
